# revision 21
# baseline (speedup 1.0000x reference)
"""FALCON ObjectSomeValuesFrom forward kernel for Trainium2 (8 NeuronCores).

Math (reference):
    e_all = concat(e_table, anon_e_emb)            # [n, d], n=1024, d=128
    Wl, Wr = W0[:, :d], W0[:, d:]
    c_fs  = sigmoid(leaky(c@Wl.T + e_all@Wr.T + b0) @ W1 + b1)        # [n]
    left  = (e_all + r) @ Wl.T ; rp = e_all @ Wr.T + b0
    z_ij  = leaky(left_i + rp_j) @ W1                                  # [n, n]
    out_i = max_j sigmoid(z_ij + b1) * c_fs[j]

Algorithm (quantized interpolation): with leaky(x) = 0.1 x + 0.9 relu(x),
    z_ij = 0.1 (lin_i + lin_j) + sum_k w9_k relu(L_ik + rp_jk),  w9 = 0.9 W1.
Clip L to [-A, A] with A >= max|rp| and correct exactly:
    relu(L + rp) = relu(clip(L) + rp) + relu(L - A)    (the last term is
    j-independent -> folded into the per-i sigmoid bias).
Quantize clip(L) on a Q-level grid l_q, piecewise-LINEAR interp in L:
    relu(L + rp) ~= sum_q hat((L - lo)/h - q) * relu(l_q + rp)
so the [n, n, d] relu tensor is replaced by Q relu tables G_q = relu(rp + l_q)
([d, n] each, built by DVE/ACT) and interpolation weights
S_q[k, i] = w9_k hat(B_ik - q) ([d, n_i]; ACT Abs + 2 cheap DVE ops per q),
contracted on the PE: z_relu = sum_q S_q^T @ G_q (PSUM accumulation over q,
4-strip concurrent matmuls).  Elementwise work drops from n_i to Q tiles.

The output column [IPC, 1] is PE-transposed to a [1, IPC] row before the
store DMA (a partition-strided 4 B/line DMA costs ~8 us; one 512 B line is
cheap).

Sharding: i-rows split across 8 cores; e_table/weights/embeddings replicated;
final max over j is local per core.
"""

import numpy as np
import ml_dtypes

N = 1024
D = 128
NCORES = 8
IPC = N // NCORES  # i rows per core = 128
H = 512            # PSUM bank free size (fp32)

Q = 16             # interpolation levels
A_CLIP = 0.5       # L clip range; exact correction term added for L > A
LO = -A_CLIP
HSTEP = 2.0 * A_CLIP / (Q - 1)

N_GACT = 5         # G tiles built by ACT (first N_GACT q's; rest on DVE)

_PROGRAM_CACHE: dict = {}

# bf16 pack layout (columns):
#   e_allT[1024] | wlT[128] | wrT[128] | w1rep01[128] |
#   rc | b0 | w9n | u01 | ch9 | c | w1 | w9 | w01 | lq[Q] | negq[Q] |
#   e_myT[128]
_C0 = N + 3 * D
_NSCAL = 9
_CLQ = _C0 + _NSCAL
_CNQ = _CLQ + Q
_CMY = _CNQ + Q
_BF_COLS = _CMY + D


def _build_program(b1f: float):
    import concourse.bacc as bacc
    import concourse.mybir as mybir
    import concourse.tile as tile

    f32 = mybir.dt.float32
    bf16 = mybir.dt.bfloat16
    A_OP = mybir.AluOpType
    AF = mybir.ActivationFunctionType

    nc = bacc.Bacc(None, target_bir_lowering=False, name="falcon_fwd")

    d_bf = nc.dram_tensor("bf_pack", [D, _BF_COLS], bf16, kind="ExternalInput")
    d_rows = nc.dram_tensor("rows", [1, D], f32, kind="ExternalInput")
    d_ident = nc.dram_tensor("ident", [D, D], f32, kind="ExternalInput")
    d_out = nc.dram_tensor("out", [1, IPC], f32, kind="ExternalOutput")

    with tile.TileContext(nc) as tc:
        with (
            tc.tile_pool(name="const", bufs=1) as const,
            tc.tile_pool(name="big", bufs=1) as big,
            tc.tile_pool(name="sw", bufs=4) as sw,
            tc.tile_pool(name="hold", bufs=1) as hold,
            tc.tile_pool(name="ps", bufs=3, space="PSUM") as ps,
            tc.tile_pool(name="psz", bufs=2, space="PSUM") as psz,
            tc.tile_pool(name="psc", bufs=2, space="PSUM") as psc,
        ):
            # ---- input DMAs: weights/cols tail first, then e_allT -----
            bf = big.tile([D, _BF_COLS], bf16)
            rows_raw = const.tile([1, D], f32)
            ident = const.tile([D, D], f32)
            nc.sync.dma_start(rows_raw[:], d_rows[:])
            nc.sync.dma_start(bf[:, N:], d_bf[:, N:])  # weights + cols + e_myT
            nc.sync.dma_start(bf[:, :H], d_bf[:, :H])
            nc.sync.dma_start(bf[:, H:N], d_bf[:, H:N])
            nc.sync.dma_start(ident[:], d_ident[:])

            # dummy sigmoid first: forces the one ACT table set that
            # contains {sigmoid, abs, relu, copy} to load exactly once
            dum = const.tile([1, 1], f32)
            nc.scalar.activation(dum[:], rows_raw[0:1, 0:1], AF.Sigmoid)

            # funnel the scalar-pointer columns through one DVE copy
            # (bf16 in the DMA pack for wide lines; f32 on-chip for ptr ops)
            colsB = const.tile([D, _NSCAL + 2 * Q], f32)
            nc.vector.tensor_copy(colsB[:], bf[:, _C0:_CMY])
            rowsS = const.tile([1, D], f32)
            nc.vector.tensor_copy(rowsS[:], rows_raw[:])

            rc = colsB[:, 0:1]
            b0c = colsB[:, 1:2]
            w9nc = colsB[:, 2:3]
            # matmul rhs columns must be bf16 -> slice the DMA'd pack
            u01b = bf[:, _C0 + 3 : _C0 + 4]
            ch9b = bf[:, _C0 + 4 : _C0 + 5]
            cb = bf[:, _C0 + 5 : _C0 + 6]
            w1b = bf[:, _C0 + 6 : _C0 + 7]
            w9b = bf[:, _C0 + 7 : _C0 + 8]
            w01b = bf[:, _C0 + 8 : _C0 + 9]
            lqc = lambda q: colsB[:, _NSCAL + q : _NSCAL + q + 1]
            nqc = lambda q: colsB[:, _NSCAL + Q + q : _NSCAL + Q + q + 1]
            eallT = bf[:, :N]
            wlT = bf[:, N : N + D]
            wrT = bf[:, N + D : N + 2 * D]
            w1rep = bf[:, N + 2 * D : N + 3 * D]
            emyT = bf[:, _CMY:]
            ones_row = rowsS[:, :]

            # ---- prologue -------------------------------------------
            # er_myT = e_myT + r (bf16), left via PE, B = clip((L-lo)/h)
            er_myT = const.tile([D, IPC], bf16)
            nc.vector.tensor_scalar(er_myT[:], emyT, rc, None, A_OP.add)
            left_ps = ps.tile([D, IPC], f32, tag="ps")
            nc.tensor.matmul(left_ps[:], wlT, er_myT[:], start=True, stop=True)
            B1 = const.tile([D, IPC], f32)
            nc.vector.tensor_scalar(
                B1[:], left_ps[:], 1.0 / HSTEP, -LO / HSTEP, A_OP.mult, A_OP.add
            )
            B = const.tile([D, IPC], f32)
            nc.vector.tensor_scalar(B[:], B1[:], 0.0, Q - 1.0, A_OP.max, A_OP.min)
            corr_t = const.tile([D, IPC], bf16)
            nc.vector.tensor_scalar(
                corr_t[:], B1[:], Q - 1.0, 0.0, A_OP.subtract, A_OP.max
            )
            # bias accumulation: 0.1*lin_i + corr_i  (then + b1)
            lini_ps = ps.tile([IPC, 1], f32, tag="ps")
            nc.tensor.matmul(lini_ps[:], er_myT[:], u01b, start=True, stop=False)
            nc.tensor.matmul(lini_ps[:], corr_t[:], ch9b, start=False, stop=True)
            biasvec = const.tile([IPC, 1], f32)
            nc.vector.tensor_scalar(biasvec[:], lini_ps[:], b1f, None, A_OP.add)

            # rbT = bf16(Wr @ e_allT + b0)
            rbT = big.tile([D, N], bf16)
            for hh in range(2):
                sl = slice(hh * H, (hh + 1) * H)
                rp_ps = ps.tile([D, H], f32, tag="ps")
                nc.tensor.matmul(rp_ps[:], wrT, eallT[:, sl], start=True, stop=True)
                nc.vector.tensor_scalar(rbT[:, sl], rp_ps[:], b0c, None, A_OP.add)

            # ---- main loop over interpolation levels ----------------
            # t1 = |B - q| for all q upfront on ACT so DVE never stalls on
            # the cross-engine dependency; G tiles split ACT/DVE; the
            # stop-q (15) G is produced early so the last accumulation has
            # no wait.
            z0 = psz.tile([D, H], f32, tag="z")
            z1 = psz.tile([D, H], f32, tag="z")

            t1s = []
            for q in range(Q):
                t1 = hold.tile([D, IPC], bf16, tag=f"t1_{q}")
                nc.scalar.activation(t1[:], B[:], AF.Abs, bias=nqc(q), scale=1.0)
                t1s.append(t1)

            # ---- c-branch: c_fs over all j --------------------------
            cl_ps = ps.tile([D, 1], f32, tag="ps")
            nc.tensor.matmul(cl_ps[:], wlT, cb, start=True, stop=True)
            cl = const.tile([D, 1], f32)
            nc.scalar.copy(cl[:], cl_ps[:])
            clb = const.tile([D, 1], bf16)
            nc.vector.tensor_copy(clb[:], cl_ps[:])
            clw_ps = ps.tile([1, 1], f32, tag="ps")
            nc.tensor.matmul(clw_ps[:], clb[:], w1b, start=True, stop=True)
            bc = const.tile([1, 1], f32)
            nc.vector.tensor_scalar(bc[:], clw_ps[:], 0.1, b1f, A_OP.mult, A_OP.add)

            cfs_row = const.tile([1, N], f32)
            Ac = big.tile([D, N], bf16)
            nc.scalar.activation(Ac[:], rbT[:], AF.Relu, bias=cl[:], scale=1.0)
            crep = []
            for hh in range(2):
                sl = slice(hh * H, (hh + 1) * H)
                zc_ps = ps.tile([1, H], f32, tag="ps")
                nc.tensor.matmul(zc_ps[:], w9b, Ac[:, sl], start=True, stop=False)
                nc.tensor.matmul(zc_ps[:], w01b, rbT[:, sl], start=False, stop=True)
                nc.scalar.activation(
                    cfs_row[:, sl], zc_ps[:], AF.Sigmoid, bias=bc[:], scale=1.0
                )
                cr = psc.tile([D, H], f32, tag="crep")
                nc.tensor.matmul(
                    cr[:], ones_row, cfs_row[0:1, sl], start=True, stop=True
                )
                crep.append(cr)


            GACT = set(range(9, 9 + N_GACT))          # ACT-built G levels
            gdve_order = [0, Q - 1] + [q for q in range(1, Q - 1) if q not in GACT]
            Gs = {}
            for q in GACT:
                G = hold.tile([D, N], bf16, tag=f"G_{q}")
                nc.scalar.activation(G[:], rbT[:], AF.Relu, bias=lqc(q), scale=1.0)
                Gs[q] = G

            def emit_sq(q):
                t2 = sw.tile([D, IPC], bf16, tag="t2")
                nc.vector.tensor_scalar(
                    t2[:], t1s[q][:], 1.0, 0.0, A_OP.subtract, A_OP.min
                )
                Sq = hold.tile([D, IPC], bf16, tag=f"Sq_{q}")
                nc.vector.tensor_scalar(Sq[:], t2[:], w9nc, None, A_OP.mult)
                return Sq

            def emit_gdve(q):
                G = hold.tile([D, N], bf16, tag=f"G_{q}")
                lq = LO + q * HSTEP
                nc.vector.tensor_scalar(
                    G[:], rbT[:], float(lq), 0.0, A_OP.add, A_OP.max
                )
                Gs[q] = G

            Sqs = {}
            gd = iter(gdve_order)
            for q in range(Q):
                Sqs[q] = emit_sq(q)
                nq = next(gd, None)
                if nq is not None:
                    emit_gdve(nq)

            for q in range(Q):
                st = q == 0
                sp = q == Q - 1
                nc.tensor.matmul(z0[:], Sqs[q][:], Gs[q][:, :H], start=st, stop=sp)
                nc.tensor.matmul(z1[:], Sqs[q][:], Gs[q][:, H:], start=st, stop=sp)
                if q == 0:
                    # fold 0.1*lin_j into every row (PSUM accumulation is
                    # order-independent)
                    nc.tensor.matmul(z0[:], w1rep, rbT[:, :H], start=False, stop=False)
                    nc.tensor.matmul(z1[:], w1rep, rbT[:, H:], start=False, stop=False)

            # ---- epilogue -------------------------------------------
            rfs = big.tile([D, N], f32)
            prod = big.tile([D, N], f32)
            outc2 = const.tile([IPC, 2], f32)
            for hh, zb in ((0, z0), (1, z1)):
                sl = slice(hh * H, (hh + 1) * H)
                nc.scalar.activation(
                    rfs[:, sl], zb[:], AF.Sigmoid, bias=biasvec[:], scale=1.0
                )
                nc.vector.tensor_tensor(
                    prod[:, sl], rfs[:, sl], crep[hh][:], A_OP.mult
                )
                nc.vector.tensor_reduce(
                    outc2[:, hh : hh + 1],
                    prod[:, sl],
                    axis=mybir.AxisListType.X,
                    op=A_OP.max,
                )
            outc = const.tile([IPC, 1], f32)
            nc.vector.tensor_tensor(
                outc[:], outc2[:, 0:1], outc2[:, 1:2], A_OP.max
            )
            # transpose [IPC, 1] -> [1, IPC] so the output DMA is one line
            orow_ps = ps.tile([1, IPC], f32, tag="ps")
            nc.tensor.matmul(orow_ps[:], outc[:], ident[:], start=True, stop=True)
            orow = const.tile([1, IPC], f32)
            nc.scalar.copy(orow[:], orow_ps[:])
            nc.sync.dma_start(d_out[:], orow[:])

    return nc


def _host_prep(anon_e_emb, e_table, c_emb, r_emb, W0, b0, W1, b1):
    f = np.float32
    bft = ml_dtypes.bfloat16
    anon_e_emb = np.asarray(anon_e_emb, f)
    e_table = np.asarray(e_table, f)
    c_emb = np.asarray(c_emb, f)
    r_emb = np.asarray(r_emb, f)
    W0 = np.asarray(W0, f)
    b0 = np.asarray(b0, f)
    W1 = np.asarray(W1, f)
    b1 = np.asarray(b1, f)

    Wl = W0[:, :D]
    e_all = np.concatenate([e_table, anon_e_emb], axis=0)  # [N, D]
    e_allT = np.ascontiguousarray(e_all.T)  # [D, N]

    bf_base = np.zeros((D, _BF_COLS), bft)
    bf_base[:, :N] = e_allT.astype(bft)
    bf_base[:, N : N + D] = Wl.T.astype(bft)
    bf_base[:, N + D : N + 2 * D] = W0[:, D:].T.astype(bft)
    bf_base[:, N + 2 * D : N + 3 * D] = np.tile(
        (0.1 * W1).astype(bft)[:, None], (1, D)
    )
    bf_base[:, _C0 + 0] = r_emb.astype(bft)
    bf_base[:, _C0 + 1] = b0.astype(bft)
    bf_base[:, _C0 + 2] = (-0.9 * W1).astype(bft)
    bf_base[:, _C0 + 3] = (0.1 * (W1 @ Wl)).astype(bft)
    bf_base[:, _C0 + 4] = (HSTEP * 0.9 * W1).astype(bft)
    bf_base[:, _C0 + 5] = c_emb.astype(bft)
    bf_base[:, _C0 + 6] = W1.astype(bft)
    bf_base[:, _C0 + 7] = (0.9 * W1).astype(bft)
    bf_base[:, _C0 + 8] = (0.1 * W1).astype(bft)
    for q in range(Q):
        bf_base[:, _CLQ + q] = np.float32(LO + q * HSTEP).astype(bft)
        bf_base[:, _CNQ + q] = np.float32(-q).astype(bft)

    rows = np.ones((1, D), f)
    ident = np.eye(D, dtype=f)
    b1f = float(b1[0])

    in_maps = []
    for c in range(NCORES):
        bf_pack = bf_base.copy()
        bf_pack[:, _CMY:] = e_allT[:, c * IPC : (c + 1) * IPC].astype(bft)
        in_maps.append({"bf_pack": bf_pack, "rows": rows, "ident": ident})
    return in_maps, b1f


def _install_ntff_shim():
    """Provide antenv.axon_hooks (missing in this image) so that
    run_bass_kernel_spmd(trace=True) can collect NTFF profiles."""
    import sys
    import types

    if "antenv.axon_hooks" in sys.modules:
        return
    try:
        import antenv
        from trn_agent_boot.trn_boot import _ntff_profile_via_ctypes
    except ImportError:
        return
    mod = types.ModuleType("antenv.axon_hooks")
    state = {"hook": None}
    mod.set_axon_ntff_profile_hook = lambda h: state.__setitem__("hook", h)
    mod.get_axon_ntff_profile_hook = lambda: state["hook"]
    sys.modules["antenv.axon_hooks"] = mod
    antenv.axon_hooks = mod
    try:
        mod.set_axon_ntff_profile_hook(
            _ntff_profile_via_ctypes("/opt/axon/libaxon_pjrt.so")
        )
    except Exception:
        pass


def kernel_ex(inputs: dict, trace: bool = False):
    """Run on 8 NeuronCores; returns (out [N] float32, BassKernelResults)."""
    from concourse.bass_utils import run_bass_kernel_spmd

    if trace:
        _install_ntff_shim()

    in_maps, b1f = _host_prep(**inputs)
    key = (round(b1f, 10),)
    nc = _PROGRAM_CACHE.get(key)
    if nc is None:
        nc = _build_program(b1f)
        nc.finalize()
        _PROGRAM_CACHE[key] = nc

    res = run_bass_kernel_spmd(
        nc, in_maps, core_ids=list(range(NCORES)), trace=trace
    )
    out = np.concatenate(
        [
            np.asarray(res.results[c]["out"], np.float32).reshape(IPC)
            for c in range(NCORES)
        ]
    )
    return out, res


def kernel(**inputs) -> np.ndarray:
    out, _ = kernel_ex(inputs, trace=False)
    return out


# revision 22
# speedup vs baseline: 1.0211x; 1.0211x over previous
"""FALCON ObjectSomeValuesFrom forward kernel for Trainium2 (8 NeuronCores).

Math (reference):
    e_all = concat(e_table, anon_e_emb)            # [n, d], n=1024, d=128
    Wl, Wr = W0[:, :d], W0[:, d:]
    c_fs  = sigmoid(leaky(c@Wl.T + e_all@Wr.T + b0) @ W1 + b1)        # [n]
    left  = (e_all + r) @ Wl.T ; rp = e_all @ Wr.T + b0
    z_ij  = leaky(left_i + rp_j) @ W1                                  # [n, n]
    out_i = max_j sigmoid(z_ij + b1) * c_fs[j]

Algorithm (quantized interpolation): with leaky(x) = 0.1 x + 0.9 relu(x),
    z_ij = 0.1 (lin_i + lin_j) + sum_k w9_k relu(L_ik + rp_jk),  w9 = 0.9 W1.
Clip L to [-A, A] with A >= max|rp| and correct exactly:
    relu(L + rp) = relu(clip(L) + rp) + relu(L - A)    (the last term is
    j-independent -> folded into the per-i sigmoid bias).
Quantize clip(L) on a Q-level grid l_q, piecewise-LINEAR interp in L:
    relu(L + rp) ~= sum_q hat((L - lo)/h - q) * relu(l_q + rp)
so the [n, n, d] relu tensor is replaced by Q relu tables G_q = relu(rp + l_q)
([d, n] each, built by DVE/ACT) and interpolation weights
S_q[k, i] = w9_k hat(B_ik - q) ([d, n_i]; ACT Abs + 2 cheap DVE ops per q),
contracted on the PE: z_relu = sum_q S_q^T @ G_q (PSUM accumulation over q,
4-strip concurrent matmuls).  Elementwise work drops from n_i to Q tiles.

The output column [IPC, 1] is PE-transposed to a [1, IPC] row before the
store DMA (a partition-strided 4 B/line DMA costs ~8 us; one 512 B line is
cheap).

Sharding: i-rows split across 8 cores; e_table/weights/embeddings replicated;
final max over j is local per core.
"""

import numpy as np
import ml_dtypes

N = 1024
D = 128
NCORES = 8
IPC = N // NCORES  # i rows per core = 128
H = 512            # PSUM bank free size (fp32)

Q = 16             # interpolation levels
A_CLIP = 0.5       # L clip range; exact correction term added for L > A
LO = -A_CLIP
HSTEP = 2.0 * A_CLIP / (Q - 1)

N_GACT = 5         # G tiles built by ACT (first N_GACT q's; rest on DVE)

_PROGRAM_CACHE: dict = {}

# bf16 pack layout (columns):
#   e_allT[1024] | wlT[128] | wrT[128] | w1rep01[128] |
#   rc | b0 | w9n | u01 | ch9 | c | w1 | w9 | w01 | lq[Q] | negq[Q] |
#   e_myT[128]
_C0 = N + 3 * D
_NSCAL = 9
_CLQ = _C0 + _NSCAL
_CNQ = _CLQ + Q
_CMY = _CNQ + Q
_BF_COLS = _CMY + D


def _build_program(b1f: float):
    import concourse.bacc as bacc
    import concourse.mybir as mybir
    import concourse.tile as tile

    f32 = mybir.dt.float32
    bf16 = mybir.dt.bfloat16
    A_OP = mybir.AluOpType
    AF = mybir.ActivationFunctionType

    nc = bacc.Bacc(None, target_bir_lowering=False, name="falcon_fwd")

    d_bf = nc.dram_tensor("bf_pack", [D, _BF_COLS], bf16, kind="ExternalInput")
    d_rows = nc.dram_tensor("rows", [1, D], f32, kind="ExternalInput")
    d_ident = nc.dram_tensor("ident", [D, D], f32, kind="ExternalInput")
    d_out = nc.dram_tensor("out", [1, IPC], f32, kind="ExternalOutput")

    with tile.TileContext(nc) as tc:
        with (
            tc.tile_pool(name="const", bufs=1) as const,
            tc.tile_pool(name="big", bufs=1) as big,
            tc.tile_pool(name="sw", bufs=4) as sw,
            tc.tile_pool(name="hold", bufs=1) as hold,
            tc.tile_pool(name="ps", bufs=3, space="PSUM") as ps,
            tc.tile_pool(name="psz", bufs=2, space="PSUM") as psz,
            tc.tile_pool(name="psc", bufs=2, space="PSUM") as psc,
        ):
            # ---- input DMAs: weights/cols tail first, then e_allT -----
            bf = big.tile([D, _BF_COLS], bf16)
            rows_raw = const.tile([1, D], f32)
            ident = const.tile([D, D], f32)
            nc.sync.dma_start(bf[:, N:], d_bf[:, N:])  # weights + cols + e_myT
            nc.sync.dma_start(bf[:, :H], d_bf[:, :H])
            nc.sync.dma_start(bf[:, H:N], d_bf[:, H:N])
            nc.sync.dma_start(rows_raw[:], d_rows[:])
            nc.sync.dma_start(ident[:], d_ident[:])

            # dummy sigmoid first: forces the one ACT table set that
            # contains {sigmoid, abs, relu, copy} to load exactly once
            dum = const.tile([1, 1], f32)
            nc.vector.memset(dum[:], 0.0)
            nc.scalar.activation(dum[:], dum[:], AF.Sigmoid)

            # funnel the scalar-pointer columns through one DVE copy
            # (bf16 in the DMA pack for wide lines; f32 on-chip for ptr ops)
            colsB = const.tile([D, _NSCAL + 2 * Q], f32)
            nc.vector.tensor_copy(colsB[:], bf[:, _C0:_CMY])
            rowsS = const.tile([1, D], f32)
            nc.vector.tensor_copy(rowsS[:], rows_raw[:])

            rc = colsB[:, 0:1]
            b0c = colsB[:, 1:2]
            w9nc = colsB[:, 2:3]
            # matmul rhs columns must be bf16 -> slice the DMA'd pack
            u01b = bf[:, _C0 + 3 : _C0 + 4]
            ch9b = bf[:, _C0 + 4 : _C0 + 5]
            cb = bf[:, _C0 + 5 : _C0 + 6]
            w1b = bf[:, _C0 + 6 : _C0 + 7]
            w9b = bf[:, _C0 + 7 : _C0 + 8]
            w01b = bf[:, _C0 + 8 : _C0 + 9]
            lqc = lambda q: colsB[:, _NSCAL + q : _NSCAL + q + 1]
            nqc = lambda q: colsB[:, _NSCAL + Q + q : _NSCAL + Q + q + 1]
            eallT = bf[:, :N]
            wlT = bf[:, N : N + D]
            wrT = bf[:, N + D : N + 2 * D]
            w1rep = bf[:, N + 2 * D : N + 3 * D]
            emyT = bf[:, _CMY:]
            ones_row = rowsS[:, :]

            # ---- prologue -------------------------------------------
            # er_myT = e_myT + r (bf16), left via PE, B = clip((L-lo)/h)
            er_myT = const.tile([D, IPC], bf16)
            nc.vector.tensor_scalar(er_myT[:], emyT, rc, None, A_OP.add)
            left_ps = ps.tile([D, IPC], f32, tag="ps")
            nc.tensor.matmul(left_ps[:], wlT, er_myT[:], start=True, stop=True)
            B1 = const.tile([D, IPC], f32)
            nc.vector.tensor_scalar(
                B1[:], left_ps[:], 1.0 / HSTEP, -LO / HSTEP, A_OP.mult, A_OP.add
            )
            B = const.tile([D, IPC], f32)
            nc.vector.tensor_scalar(B[:], B1[:], 0.0, Q - 1.0, A_OP.max, A_OP.min)
            corr_t = const.tile([D, IPC], bf16)
            nc.vector.tensor_scalar(
                corr_t[:], B1[:], Q - 1.0, 0.0, A_OP.subtract, A_OP.max
            )
            # bias accumulation: 0.1*lin_i + corr_i  (then + b1)
            lini_ps = ps.tile([IPC, 1], f32, tag="ps")
            nc.tensor.matmul(lini_ps[:], er_myT[:], u01b, start=True, stop=False)
            nc.tensor.matmul(lini_ps[:], corr_t[:], ch9b, start=False, stop=True)
            biasvec = const.tile([IPC, 1], f32)
            nc.vector.tensor_scalar(biasvec[:], lini_ps[:], b1f, None, A_OP.add)

            # rbT = bf16(Wr @ e_allT + b0)
            rbT = big.tile([D, N], bf16)
            for hh in range(2):
                sl = slice(hh * H, (hh + 1) * H)
                rp_ps = ps.tile([D, H], f32, tag="ps")
                nc.tensor.matmul(rp_ps[:], wrT, eallT[:, sl], start=True, stop=True)
                nc.vector.tensor_scalar(rbT[:, sl], rp_ps[:], b0c, None, A_OP.add)

            # ---- main loop over interpolation levels ----------------
            # t1 = |B - q| for all q upfront on ACT so DVE never stalls on
            # the cross-engine dependency; G tiles split ACT/DVE; the
            # stop-q (15) G is produced early so the last accumulation has
            # no wait.
            z0 = psz.tile([D, H], f32, tag="z")
            z1 = psz.tile([D, H], f32, tag="z")

            t1s = []
            for q in range(Q):
                t1 = hold.tile([D, IPC], bf16, tag=f"t1_{q}")
                nc.scalar.activation(t1[:], B[:], AF.Abs, bias=nqc(q), scale=1.0)
                t1s.append(t1)

            # ---- c-branch: c_fs over all j --------------------------
            cl_ps = ps.tile([D, 1], f32, tag="ps")
            nc.tensor.matmul(cl_ps[:], wlT, cb, start=True, stop=True)
            cl = const.tile([D, 1], f32)
            nc.scalar.copy(cl[:], cl_ps[:])
            clb = const.tile([D, 1], bf16)
            nc.vector.tensor_copy(clb[:], cl_ps[:])
            clw_ps = ps.tile([1, 1], f32, tag="ps")
            nc.tensor.matmul(clw_ps[:], clb[:], w1b, start=True, stop=True)
            bc = const.tile([1, 1], f32)
            nc.vector.tensor_scalar(bc[:], clw_ps[:], 0.1, b1f, A_OP.mult, A_OP.add)

            cfs_row = const.tile([1, N], f32)
            Ac = big.tile([D, N], bf16)
            nc.scalar.activation(Ac[:], rbT[:], AF.Relu, bias=cl[:], scale=1.0)
            crep = []
            for hh in range(2):
                sl = slice(hh * H, (hh + 1) * H)
                zc_ps = ps.tile([1, H], f32, tag="ps")
                nc.tensor.matmul(zc_ps[:], w9b, Ac[:, sl], start=True, stop=False)
                nc.tensor.matmul(zc_ps[:], w01b, rbT[:, sl], start=False, stop=True)
                nc.scalar.activation(
                    cfs_row[:, sl], zc_ps[:], AF.Sigmoid, bias=bc[:], scale=1.0
                )
                cr = psc.tile([D, H], f32, tag="crep")
                nc.tensor.matmul(
                    cr[:], ones_row, cfs_row[0:1, sl], start=True, stop=True
                )
                crep.append(cr)


            GACT = set(range(9, 9 + N_GACT))          # ACT-built G levels
            gdve_order = [0, Q - 1] + [q for q in range(1, Q - 1) if q not in GACT]
            Gs = {}
            for q in GACT:
                G = hold.tile([D, N], bf16, tag=f"G_{q}")
                nc.scalar.activation(G[:], rbT[:], AF.Relu, bias=lqc(q), scale=1.0)
                Gs[q] = G

            def emit_sq(q):
                t2 = sw.tile([D, IPC], bf16, tag="t2")
                nc.vector.tensor_scalar(
                    t2[:], t1s[q][:], 1.0, 0.0, A_OP.subtract, A_OP.min
                )
                Sq = hold.tile([D, IPC], bf16, tag=f"Sq_{q}")
                nc.vector.tensor_scalar(Sq[:], t2[:], w9nc, None, A_OP.mult)
                return Sq

            def emit_gdve(q):
                G = hold.tile([D, N], bf16, tag=f"G_{q}")
                lq = LO + q * HSTEP
                nc.vector.tensor_scalar(
                    G[:], rbT[:], float(lq), 0.0, A_OP.add, A_OP.max
                )
                Gs[q] = G

            Sqs = {}
            gd = iter(gdve_order)
            for q in range(Q):
                Sqs[q] = emit_sq(q)
                nq = next(gd, None)
                if nq is not None:
                    emit_gdve(nq)

            for q in range(Q):
                st = q == 0
                sp = q == Q - 1
                nc.tensor.matmul(z0[:], Sqs[q][:], Gs[q][:, :H], start=st, stop=sp)
                nc.tensor.matmul(z1[:], Sqs[q][:], Gs[q][:, H:], start=st, stop=sp)
                if q == 0:
                    # fold 0.1*lin_j into every row (PSUM accumulation is
                    # order-independent)
                    nc.tensor.matmul(z0[:], w1rep, rbT[:, :H], start=False, stop=False)
                    nc.tensor.matmul(z1[:], w1rep, rbT[:, H:], start=False, stop=False)

            # ---- epilogue -------------------------------------------
            rfs = big.tile([D, N], f32)
            prod = big.tile([D, N], f32)
            outc2 = const.tile([IPC, 2], f32)
            for hh, zb in ((0, z0), (1, z1)):
                sl = slice(hh * H, (hh + 1) * H)
                nc.scalar.activation(
                    rfs[:, sl], zb[:], AF.Sigmoid, bias=biasvec[:], scale=1.0
                )
                nc.vector.tensor_tensor(
                    prod[:, sl], rfs[:, sl], crep[hh][:], A_OP.mult
                )
                nc.vector.tensor_reduce(
                    outc2[:, hh : hh + 1],
                    prod[:, sl],
                    axis=mybir.AxisListType.X,
                    op=A_OP.max,
                )
            outc = const.tile([IPC, 1], f32)
            nc.vector.tensor_tensor(
                outc[:], outc2[:, 0:1], outc2[:, 1:2], A_OP.max
            )
            # transpose [IPC, 1] -> [1, IPC] so the output DMA is one line
            orow_ps = ps.tile([1, IPC], f32, tag="ps")
            nc.tensor.matmul(orow_ps[:], outc[:], ident[:], start=True, stop=True)
            orow = const.tile([1, IPC], f32)
            nc.scalar.copy(orow[:], orow_ps[:])
            nc.sync.dma_start(d_out[:], orow[:])

    return nc


def _host_prep(anon_e_emb, e_table, c_emb, r_emb, W0, b0, W1, b1):
    f = np.float32
    bft = ml_dtypes.bfloat16
    anon_e_emb = np.asarray(anon_e_emb, f)
    e_table = np.asarray(e_table, f)
    c_emb = np.asarray(c_emb, f)
    r_emb = np.asarray(r_emb, f)
    W0 = np.asarray(W0, f)
    b0 = np.asarray(b0, f)
    W1 = np.asarray(W1, f)
    b1 = np.asarray(b1, f)

    Wl = W0[:, :D]
    e_all = np.concatenate([e_table, anon_e_emb], axis=0)  # [N, D]
    e_allT = np.ascontiguousarray(e_all.T)  # [D, N]

    bf_base = np.zeros((D, _BF_COLS), bft)
    bf_base[:, :N] = e_allT.astype(bft)
    bf_base[:, N : N + D] = Wl.T.astype(bft)
    bf_base[:, N + D : N + 2 * D] = W0[:, D:].T.astype(bft)
    bf_base[:, N + 2 * D : N + 3 * D] = np.tile(
        (0.1 * W1).astype(bft)[:, None], (1, D)
    )
    bf_base[:, _C0 + 0] = r_emb.astype(bft)
    bf_base[:, _C0 + 1] = b0.astype(bft)
    bf_base[:, _C0 + 2] = (-0.9 * W1).astype(bft)
    bf_base[:, _C0 + 3] = (0.1 * (W1 @ Wl)).astype(bft)
    bf_base[:, _C0 + 4] = (HSTEP * 0.9 * W1).astype(bft)
    bf_base[:, _C0 + 5] = c_emb.astype(bft)
    bf_base[:, _C0 + 6] = W1.astype(bft)
    bf_base[:, _C0 + 7] = (0.9 * W1).astype(bft)
    bf_base[:, _C0 + 8] = (0.1 * W1).astype(bft)
    for q in range(Q):
        bf_base[:, _CLQ + q] = np.float32(LO + q * HSTEP).astype(bft)
        bf_base[:, _CNQ + q] = np.float32(-q).astype(bft)

    rows = np.ones((1, D), f)
    ident = np.eye(D, dtype=f)
    b1f = float(b1[0])

    in_maps = []
    for c in range(NCORES):
        bf_pack = bf_base.copy()
        bf_pack[:, _CMY:] = e_allT[:, c * IPC : (c + 1) * IPC].astype(bft)
        in_maps.append({"bf_pack": bf_pack, "rows": rows, "ident": ident})
    return in_maps, b1f


def _install_ntff_shim():
    """Provide antenv.axon_hooks (missing in this image) so that
    run_bass_kernel_spmd(trace=True) can collect NTFF profiles."""
    import sys
    import types

    if "antenv.axon_hooks" in sys.modules:
        return
    try:
        import antenv
        from trn_agent_boot.trn_boot import _ntff_profile_via_ctypes
    except ImportError:
        return
    mod = types.ModuleType("antenv.axon_hooks")
    state = {"hook": None}
    mod.set_axon_ntff_profile_hook = lambda h: state.__setitem__("hook", h)
    mod.get_axon_ntff_profile_hook = lambda: state["hook"]
    sys.modules["antenv.axon_hooks"] = mod
    antenv.axon_hooks = mod
    try:
        mod.set_axon_ntff_profile_hook(
            _ntff_profile_via_ctypes("/opt/axon/libaxon_pjrt.so")
        )
    except Exception:
        pass


def kernel_ex(inputs: dict, trace: bool = False):
    """Run on 8 NeuronCores; returns (out [N] float32, BassKernelResults)."""
    from concourse.bass_utils import run_bass_kernel_spmd

    if trace:
        _install_ntff_shim()

    in_maps, b1f = _host_prep(**inputs)
    key = (round(b1f, 10),)
    nc = _PROGRAM_CACHE.get(key)
    if nc is None:
        nc = _build_program(b1f)
        nc.finalize()
        _PROGRAM_CACHE[key] = nc

    res = run_bass_kernel_spmd(
        nc, in_maps, core_ids=list(range(NCORES)), trace=trace
    )
    out = np.concatenate(
        [
            np.asarray(res.results[c]["out"], np.float32).reshape(IPC)
            for c in range(NCORES)
        ]
    )
    return out, res


def kernel(**inputs) -> np.ndarray:
    out, _ = kernel_ex(inputs, trace=False)
    return out


# revision 23
# speedup vs baseline: 1.0242x; 1.0031x over previous
"""FALCON ObjectSomeValuesFrom forward kernel for Trainium2 (8 NeuronCores).

Math (reference):
    e_all = concat(e_table, anon_e_emb)            # [n, d], n=1024, d=128
    Wl, Wr = W0[:, :d], W0[:, d:]
    c_fs  = sigmoid(leaky(c@Wl.T + e_all@Wr.T + b0) @ W1 + b1)        # [n]
    left  = (e_all + r) @ Wl.T ; rp = e_all @ Wr.T + b0
    z_ij  = leaky(left_i + rp_j) @ W1                                  # [n, n]
    out_i = max_j sigmoid(z_ij + b1) * c_fs[j]

Algorithm (quantized interpolation): with leaky(x) = 0.1 x + 0.9 relu(x),
    z_ij = 0.1 (lin_i + lin_j) + sum_k w9_k relu(L_ik + rp_jk),  w9 = 0.9 W1.
Clip L to [-A, A] with A >= max|rp| and correct exactly:
    relu(L + rp) = relu(clip(L) + rp) + relu(L - A)    (the last term is
    j-independent -> folded into the per-i sigmoid bias).
Quantize clip(L) on a Q-level grid l_q, piecewise-LINEAR interp in L:
    relu(L + rp) ~= sum_q hat((L - lo)/h - q) * relu(l_q + rp)
so the [n, n, d] relu tensor is replaced by Q relu tables G_q = relu(rp + l_q)
([d, n] each, built by DVE/ACT) and interpolation weights
S_q[k, i] = w9_k hat(B_ik - q) ([d, n_i]; ACT Abs + 2 cheap DVE ops per q),
contracted on the PE: z_relu = sum_q S_q^T @ G_q (PSUM accumulation over q,
4-strip concurrent matmuls).  Elementwise work drops from n_i to Q tiles.

The output column [IPC, 1] is PE-transposed to a [1, IPC] row before the
store DMA (a partition-strided 4 B/line DMA costs ~8 us; one 512 B line is
cheap).

Sharding: i-rows split across 8 cores; e_table/weights/embeddings replicated;
final max over j is local per core.
"""

import numpy as np
import ml_dtypes

N = 1024
D = 128
NCORES = 8
IPC = N // NCORES  # i rows per core = 128
H = 512            # PSUM bank free size (fp32)

Q = 16             # interpolation levels
A_CLIP = 0.5       # L clip range; exact correction term added for L > A
LO = -A_CLIP
HSTEP = 2.0 * A_CLIP / (Q - 1)

N_GACT = 5         # G tiles built by ACT (first N_GACT q's; rest on DVE)

_PROGRAM_CACHE: dict = {}

# bf16 pack layout (columns):
#   e_allT[1024] | wlT[128] | wrT[128] | w1rep01[128] |
#   rc | b0 | w9n | u01 | ch9 | c | w1 | w9 | w01 | lq[Q] | negq[Q] |
#   e_myT[128]
_C0 = N + 3 * D
_NSCAL = 9
_CLQ = _C0 + _NSCAL
_CNQ = _CLQ + Q
_CMY = _CNQ + Q
_BF_COLS = _CMY + D


def _build_program(b1f: float):
    import concourse.bacc as bacc
    import concourse.mybir as mybir
    import concourse.tile as tile

    f32 = mybir.dt.float32
    bf16 = mybir.dt.bfloat16
    A_OP = mybir.AluOpType
    AF = mybir.ActivationFunctionType

    nc = bacc.Bacc(None, target_bir_lowering=False, name="falcon_fwd")

    d_bf = nc.dram_tensor("bf_pack", [D, _BF_COLS], bf16, kind="ExternalInput")
    d_rows = nc.dram_tensor("rows", [1, D], f32, kind="ExternalInput")
    d_ident = nc.dram_tensor("ident", [D, D], f32, kind="ExternalInput")
    d_out = nc.dram_tensor("out", [1, IPC], f32, kind="ExternalOutput")

    with tile.TileContext(nc) as tc:
        with (
            tc.tile_pool(name="const", bufs=1) as const,
            tc.tile_pool(name="big", bufs=1) as big,
            tc.tile_pool(name="sw", bufs=4) as sw,
            tc.tile_pool(name="hold", bufs=1) as hold,
            tc.tile_pool(name="ps", bufs=3, space="PSUM") as ps,
            tc.tile_pool(name="psz", bufs=2, space="PSUM") as psz,
            tc.tile_pool(name="psc", bufs=2, space="PSUM") as psc,
        ):
            # ---- input DMAs: weights/cols tail first, then e_allT -----
            bf = big.tile([D, _BF_COLS], bf16)
            rows_raw = const.tile([1, D], f32)
            ident = const.tile([D, D], f32)
            nc.sync.dma_start(bf[:, N:], d_bf[:, N:])  # weights + cols + e_myT
            nc.sync.dma_start(bf[:, :H], d_bf[:, :H])
            nc.sync.dma_start(bf[:, H:N], d_bf[:, H:N])
            nc.sync.dma_start(rows_raw[:], d_rows[:])
            nc.sync.dma_start(ident[:], d_ident[:])


            # funnel the scalar-pointer columns through one DVE copy
            # (bf16 in the DMA pack for wide lines; f32 on-chip for ptr ops)
            colsB = const.tile([D, _NSCAL + 2 * Q], f32)
            nc.vector.tensor_copy(colsB[:], bf[:, _C0:_CMY])
            rowsS = const.tile([1, D], f32)
            nc.vector.tensor_copy(rowsS[:], rows_raw[:])

            rc = colsB[:, 0:1]
            b0c = colsB[:, 1:2]
            w9nc = colsB[:, 2:3]
            # matmul rhs columns must be bf16 -> slice the DMA'd pack
            u01b = bf[:, _C0 + 3 : _C0 + 4]
            ch9b = bf[:, _C0 + 4 : _C0 + 5]
            cb = bf[:, _C0 + 5 : _C0 + 6]
            w1b = bf[:, _C0 + 6 : _C0 + 7]
            w9b = bf[:, _C0 + 7 : _C0 + 8]
            w01b = bf[:, _C0 + 8 : _C0 + 9]
            lqc = lambda q: colsB[:, _NSCAL + q : _NSCAL + q + 1]
            nqc = lambda q: colsB[:, _NSCAL + Q + q : _NSCAL + Q + q + 1]
            eallT = bf[:, :N]
            wlT = bf[:, N : N + D]
            wrT = bf[:, N + D : N + 2 * D]
            w1rep = bf[:, N + 2 * D : N + 3 * D]
            emyT = bf[:, _CMY:]
            ones_row = rowsS[:, :]

            # ---- prologue -------------------------------------------
            # er_myT = e_myT + r (bf16), left via PE, B = clip((L-lo)/h)
            er_myT = const.tile([D, IPC], bf16)
            nc.vector.tensor_scalar(er_myT[:], emyT, rc, None, A_OP.add)
            left_ps = ps.tile([D, IPC], f32, tag="ps")
            nc.tensor.matmul(left_ps[:], wlT, er_myT[:], start=True, stop=True)
            B1 = const.tile([D, IPC], f32)
            nc.vector.tensor_scalar(
                B1[:], left_ps[:], 1.0 / HSTEP, -LO / HSTEP, A_OP.mult, A_OP.add
            )
            B = const.tile([D, IPC], f32)
            nc.vector.tensor_scalar(B[:], B1[:], 0.0, Q - 1.0, A_OP.max, A_OP.min)
            corr_t = const.tile([D, IPC], bf16)
            nc.vector.tensor_scalar(
                corr_t[:], B1[:], Q - 1.0, 0.0, A_OP.subtract, A_OP.max
            )
            # bias accumulation: 0.1*lin_i + corr_i  (then + b1)
            lini_ps = ps.tile([IPC, 1], f32, tag="ps")
            nc.tensor.matmul(lini_ps[:], er_myT[:], u01b, start=True, stop=False)
            nc.tensor.matmul(lini_ps[:], corr_t[:], ch9b, start=False, stop=True)
            biasvec = const.tile([IPC, 1], f32)
            nc.vector.tensor_scalar(biasvec[:], lini_ps[:], b1f, None, A_OP.add)

            # rbT = bf16(Wr @ e_allT + b0)
            rbT = big.tile([D, N], bf16)
            for hh in range(2):
                sl = slice(hh * H, (hh + 1) * H)
                rp_ps = ps.tile([D, H], f32, tag="ps")
                nc.tensor.matmul(rp_ps[:], wrT, eallT[:, sl], start=True, stop=True)
                nc.vector.tensor_scalar(rbT[:, sl], rp_ps[:], b0c, None, A_OP.add)

            # ---- main loop over interpolation levels ----------------
            # t1 = |B - q| for all q upfront on ACT so DVE never stalls on
            # the cross-engine dependency; G tiles split ACT/DVE; the
            # stop-q (15) G is produced early so the last accumulation has
            # no wait.
            z0 = psz.tile([D, H], f32, tag="z")
            z1 = psz.tile([D, H], f32, tag="z")

            t1s = []
            for q in range(Q):
                t1 = hold.tile([D, IPC], bf16, tag=f"t1_{q}")
                nc.scalar.activation(t1[:], B[:], AF.Abs, bias=nqc(q), scale=1.0)
                t1s.append(t1)

            # ---- c-branch: c_fs over all j --------------------------
            cl_ps = ps.tile([D, 1], f32, tag="ps")
            nc.tensor.matmul(cl_ps[:], wlT, cb, start=True, stop=True)
            cl = const.tile([D, 1], f32)
            nc.scalar.copy(cl[:], cl_ps[:])
            clb = const.tile([D, 1], bf16)
            nc.vector.tensor_copy(clb[:], cl_ps[:])
            clw_ps = ps.tile([1, 1], f32, tag="ps")
            nc.tensor.matmul(clw_ps[:], clb[:], w1b, start=True, stop=True)
            bc = const.tile([1, 1], f32)
            nc.vector.tensor_scalar(bc[:], clw_ps[:], 0.1, b1f, A_OP.mult, A_OP.add)

            cfs_row = const.tile([1, N], f32)
            Ac = big.tile([D, N], bf16)
            nc.scalar.activation(Ac[:], rbT[:], AF.Relu, bias=cl[:], scale=1.0)
            crep = []
            for hh in range(2):
                sl = slice(hh * H, (hh + 1) * H)
                zc_ps = ps.tile([1, H], f32, tag="ps")
                nc.tensor.matmul(zc_ps[:], w9b, Ac[:, sl], start=True, stop=False)
                nc.tensor.matmul(zc_ps[:], w01b, rbT[:, sl], start=False, stop=True)
                nc.scalar.activation(
                    cfs_row[:, sl], zc_ps[:], AF.Sigmoid, bias=bc[:], scale=1.0
                )
                cr = psc.tile([D, H], f32, tag="crep")
                nc.tensor.matmul(
                    cr[:], ones_row, cfs_row[0:1, sl], start=True, stop=True
                )
                crep.append(cr)


            GACT = set(range(9, 9 + N_GACT))          # ACT-built G levels
            gdve_order = [0, Q - 1] + [q for q in range(1, Q - 1) if q not in GACT]
            Gs = {}
            for q in GACT:
                G = hold.tile([D, N], bf16, tag=f"G_{q}")
                nc.scalar.activation(G[:], rbT[:], AF.Relu, bias=lqc(q), scale=1.0)
                Gs[q] = G

            def emit_sq(q):
                t2 = sw.tile([D, IPC], bf16, tag="t2")
                nc.vector.tensor_scalar(
                    t2[:], t1s[q][:], 1.0, 0.0, A_OP.subtract, A_OP.min
                )
                Sq = hold.tile([D, IPC], bf16, tag=f"Sq_{q}")
                nc.vector.tensor_scalar(Sq[:], t2[:], w9nc, None, A_OP.mult)
                return Sq

            def emit_gdve(q):
                G = hold.tile([D, N], bf16, tag=f"G_{q}")
                lq = LO + q * HSTEP
                nc.vector.tensor_scalar(
                    G[:], rbT[:], float(lq), 0.0, A_OP.add, A_OP.max
                )
                Gs[q] = G

            Sqs = {}
            gd = iter(gdve_order)
            for q in range(Q):
                Sqs[q] = emit_sq(q)
                nq = next(gd, None)
                if nq is not None:
                    emit_gdve(nq)

            for q in range(Q):
                st = q == 0
                sp = q == Q - 1
                nc.tensor.matmul(z0[:], Sqs[q][:], Gs[q][:, :H], start=st, stop=sp)
                nc.tensor.matmul(z1[:], Sqs[q][:], Gs[q][:, H:], start=st, stop=sp)
                if q == 0:
                    # fold 0.1*lin_j into every row (PSUM accumulation is
                    # order-independent)
                    nc.tensor.matmul(z0[:], w1rep, rbT[:, :H], start=False, stop=False)
                    nc.tensor.matmul(z1[:], w1rep, rbT[:, H:], start=False, stop=False)

            # ---- epilogue -------------------------------------------
            rfs = big.tile([D, N], f32)
            prod = big.tile([D, N], f32)
            outc2 = const.tile([IPC, 2], f32)
            for hh, zb in ((0, z0), (1, z1)):
                sl = slice(hh * H, (hh + 1) * H)
                nc.scalar.activation(
                    rfs[:, sl], zb[:], AF.Sigmoid, bias=biasvec[:], scale=1.0
                )
                nc.vector.tensor_tensor(
                    prod[:, sl], rfs[:, sl], crep[hh][:], A_OP.mult
                )
                nc.vector.tensor_reduce(
                    outc2[:, hh : hh + 1],
                    prod[:, sl],
                    axis=mybir.AxisListType.X,
                    op=A_OP.max,
                )
            outc = const.tile([IPC, 1], f32)
            nc.vector.tensor_tensor(
                outc[:], outc2[:, 0:1], outc2[:, 1:2], A_OP.max
            )
            # transpose [IPC, 1] -> [1, IPC] so the output DMA is one line
            orow_ps = ps.tile([1, IPC], f32, tag="ps")
            nc.tensor.matmul(orow_ps[:], outc[:], ident[:], start=True, stop=True)
            orow = const.tile([1, IPC], f32)
            nc.scalar.copy(orow[:], orow_ps[:])
            nc.sync.dma_start(d_out[:], orow[:])

    return nc


def _host_prep(anon_e_emb, e_table, c_emb, r_emb, W0, b0, W1, b1):
    f = np.float32
    bft = ml_dtypes.bfloat16
    anon_e_emb = np.asarray(anon_e_emb, f)
    e_table = np.asarray(e_table, f)
    c_emb = np.asarray(c_emb, f)
    r_emb = np.asarray(r_emb, f)
    W0 = np.asarray(W0, f)
    b0 = np.asarray(b0, f)
    W1 = np.asarray(W1, f)
    b1 = np.asarray(b1, f)

    Wl = W0[:, :D]
    e_all = np.concatenate([e_table, anon_e_emb], axis=0)  # [N, D]
    e_allT = np.ascontiguousarray(e_all.T)  # [D, N]

    bf_base = np.zeros((D, _BF_COLS), bft)
    bf_base[:, :N] = e_allT.astype(bft)
    bf_base[:, N : N + D] = Wl.T.astype(bft)
    bf_base[:, N + D : N + 2 * D] = W0[:, D:].T.astype(bft)
    bf_base[:, N + 2 * D : N + 3 * D] = np.tile(
        (0.1 * W1).astype(bft)[:, None], (1, D)
    )
    bf_base[:, _C0 + 0] = r_emb.astype(bft)
    bf_base[:, _C0 + 1] = b0.astype(bft)
    bf_base[:, _C0 + 2] = (-0.9 * W1).astype(bft)
    bf_base[:, _C0 + 3] = (0.1 * (W1 @ Wl)).astype(bft)
    bf_base[:, _C0 + 4] = (HSTEP * 0.9 * W1).astype(bft)
    bf_base[:, _C0 + 5] = c_emb.astype(bft)
    bf_base[:, _C0 + 6] = W1.astype(bft)
    bf_base[:, _C0 + 7] = (0.9 * W1).astype(bft)
    bf_base[:, _C0 + 8] = (0.1 * W1).astype(bft)
    for q in range(Q):
        bf_base[:, _CLQ + q] = np.float32(LO + q * HSTEP).astype(bft)
        bf_base[:, _CNQ + q] = np.float32(-q).astype(bft)

    rows = np.ones((1, D), f)
    ident = np.eye(D, dtype=f)
    b1f = float(b1[0])

    in_maps = []
    for c in range(NCORES):
        bf_pack = bf_base.copy()
        bf_pack[:, _CMY:] = e_allT[:, c * IPC : (c + 1) * IPC].astype(bft)
        in_maps.append({"bf_pack": bf_pack, "rows": rows, "ident": ident})
    return in_maps, b1f


def _install_ntff_shim():
    """Provide antenv.axon_hooks (missing in this image) so that
    run_bass_kernel_spmd(trace=True) can collect NTFF profiles."""
    import sys
    import types

    if "antenv.axon_hooks" in sys.modules:
        return
    try:
        import antenv
        from trn_agent_boot.trn_boot import _ntff_profile_via_ctypes
    except ImportError:
        return
    mod = types.ModuleType("antenv.axon_hooks")
    state = {"hook": None}
    mod.set_axon_ntff_profile_hook = lambda h: state.__setitem__("hook", h)
    mod.get_axon_ntff_profile_hook = lambda: state["hook"]
    sys.modules["antenv.axon_hooks"] = mod
    antenv.axon_hooks = mod
    try:
        mod.set_axon_ntff_profile_hook(
            _ntff_profile_via_ctypes("/opt/axon/libaxon_pjrt.so")
        )
    except Exception:
        pass


def kernel_ex(inputs: dict, trace: bool = False):
    """Run on 8 NeuronCores; returns (out [N] float32, BassKernelResults)."""
    from concourse.bass_utils import run_bass_kernel_spmd

    if trace:
        _install_ntff_shim()

    in_maps, b1f = _host_prep(**inputs)
    key = (round(b1f, 10),)
    nc = _PROGRAM_CACHE.get(key)
    if nc is None:
        nc = _build_program(b1f)
        nc.finalize()
        _PROGRAM_CACHE[key] = nc

    res = run_bass_kernel_spmd(
        nc, in_maps, core_ids=list(range(NCORES)), trace=trace
    )
    out = np.concatenate(
        [
            np.asarray(res.results[c]["out"], np.float32).reshape(IPC)
            for c in range(NCORES)
        ]
    )
    return out, res


def kernel(**inputs) -> np.ndarray:
    out, _ = kernel_ex(inputs, trace=False)
    return out


# revision 29
# speedup vs baseline: 1.2045x; 1.1760x over previous
"""FALCON ObjectSomeValuesFrom forward kernel for Trainium2 (8 NeuronCores).

Math (reference):
    e_all = concat(e_table, anon_e_emb)            # [n, d], n=1024, d=128
    Wl, Wr = W0[:, :d], W0[:, d:]
    c_fs  = sigmoid(leaky(c@Wl.T + e_all@Wr.T + b0) @ W1 + b1)        # [n]
    left  = (e_all + r) @ Wl.T ; rp = e_all @ Wr.T + b0
    z_ij  = leaky(left_i + rp_j) @ W1                                  # [n, n]
    out_i = max_j sigmoid(z_ij + b1) * c_fs[j]

Algorithm (quantized interpolation): with leaky(x) = 0.1 x + 0.9 relu(x),
    z_ij = 0.1 (lin_i + lin_j) + sum_k w9_k relu(L_ik + rp_jk),  w9 = 0.9 W1.
Clip L to [-A, A] with A >= max|rp| and correct exactly:
    relu(L + rp) = relu(clip(L) + rp) + relu(L - A)    (the last term is
    j-independent -> folded into the per-i sigmoid bias).
Quantize clip(L) on a Q-level grid l_q, piecewise-LINEAR interp in L:
    relu(L + rp) ~= sum_q hat((L - lo)/h - q) * relu(l_q + rp)
so the [n, n, d] relu tensor is replaced by Q relu tables G_q = relu(rp + l_q)
([d, n] each, built by DVE/ACT) and interpolation weights
S_q[k, i] = w9_k hat(B_ik - q) ([d, n_i]; ACT Abs + 2 cheap DVE ops per q),
contracted on the PE: z_relu = sum_q S_q^T @ G_q (PSUM accumulation over q,
4-strip concurrent matmuls).  Elementwise work drops from n_i to Q tiles.

The output column [IPC, 1] is PE-transposed to a [1, IPC] row before the
store DMA (a partition-strided 4 B/line DMA costs ~8 us; one 512 B line is
cheap).

Sharding: i-rows split across 8 cores; e_table/weights/embeddings replicated;
final max over j is local per core.
"""

import numpy as np
import ml_dtypes

N = 1024
D = 128
NCORES = 8
IPC = N // NCORES  # i rows per core = 128
H = 512            # PSUM bank free size (fp32)

Q = 16             # interpolation levels
A_CLIP = 0.5       # L clip range; exact correction term added for L > A
LO = -A_CLIP
HSTEP = 2.0 * A_CLIP / (Q - 1)

N_GACT = 3         # G tiles built by ACT (rest on DVE)

_PROGRAM_CACHE: dict = {}

# bf16 pack layout (columns):
#   e_allT[1024] | wlT[128] | wrT[128] | w1rep01[128] |
#   rc | b0 | w9n | u01 | ch9 | c | w1 | w9 | w01 | lq[Q] | negq[Q] |
#   e_myT[128]
_C0 = N + 3 * D
_NSCAL = 9
_CLQ = _C0 + _NSCAL
_CNQ = _CLQ + Q
_CMY = _CNQ + Q
_BF_COLS = _CMY + D


def _build_program(b1f: float):
    import concourse.bacc as bacc
    import concourse.mybir as mybir
    import concourse.tile as tile

    f32 = mybir.dt.float32
    bf16 = mybir.dt.bfloat16
    A_OP = mybir.AluOpType
    AF = mybir.ActivationFunctionType

    nc = bacc.Bacc(None, target_bir_lowering=False, name="falcon_fwd")

    d_bf = nc.dram_tensor("bf_pack", [D, _BF_COLS], bf16, kind="ExternalInput")
    d_rows = nc.dram_tensor("rows", [1, D], f32, kind="ExternalInput")
    d_ident = nc.dram_tensor("ident", [D, D], f32, kind="ExternalInput")
    d_out = nc.dram_tensor("out", [1, IPC], f32, kind="ExternalOutput")

    with tile.TileContext(nc) as tc:
        with (
            tc.tile_pool(name="const", bufs=1) as const,
            tc.tile_pool(name="big", bufs=1) as big,
            tc.tile_pool(name="sw", bufs=4) as sw,
            tc.tile_pool(name="hold", bufs=1) as hold,
            tc.tile_pool(name="ps", bufs=3, space="PSUM") as ps,
            tc.tile_pool(name="psz", bufs=2, space="PSUM") as psz,
            tc.tile_pool(name="psc", bufs=2, space="PSUM") as psc,
        ):
            # ---- input DMAs: weights/cols tail first, then e_allT -----
            bf = big.tile([D, _BF_COLS], bf16)
            rows_raw = const.tile([1, D], f32)
            ident = const.tile([D, D], f32)
            nc.sync.dma_start(bf[:, N:], d_bf[:, N:])  # weights + cols + e_myT
            nc.sync.dma_start(bf[:, :H], d_bf[:, :H])
            nc.sync.dma_start(bf[:, H:N], d_bf[:, H:N])
            nc.sync.dma_start(rows_raw[:], d_rows[:])
            nc.sync.dma_start(ident[:], d_ident[:])

            # dummy sigmoid first: forces the one ACT table set that
            # contains {sigmoid, abs, relu, copy} to load exactly once
            dum = const.tile([1, 1], f32)
            nc.vector.memset(dum[:], 0.0)
            nc.scalar.activation(dum[:], dum[:], AF.Sigmoid)

            # funnel the scalar-pointer columns through one DVE copy
            # (bf16 in the DMA pack for wide lines; f32 on-chip for ptr ops)
            colsB = const.tile([D, _NSCAL + 2 * Q], f32)
            nc.vector.tensor_copy(colsB[:], bf[:, _C0:_CMY])
            rowsS = const.tile([1, D], f32)
            nc.vector.tensor_copy(rowsS[:], rows_raw[:])

            rc = colsB[:, 0:1]
            b0c = colsB[:, 1:2]
            w9nc = colsB[:, 2:3]
            # matmul rhs columns must be bf16 -> slice the DMA'd pack
            u01b = bf[:, _C0 + 3 : _C0 + 4]
            ch9b = bf[:, _C0 + 4 : _C0 + 5]
            cb = bf[:, _C0 + 5 : _C0 + 6]
            w1b = bf[:, _C0 + 6 : _C0 + 7]
            w9b = bf[:, _C0 + 7 : _C0 + 8]
            w01b = bf[:, _C0 + 8 : _C0 + 9]
            lqc = lambda q: colsB[:, _NSCAL + q : _NSCAL + q + 1]
            nqc = lambda q: colsB[:, _NSCAL + Q + q : _NSCAL + Q + q + 1]
            eallT = bf[:, :N]
            wlT = bf[:, N : N + D]
            wrT = bf[:, N + D : N + 2 * D]
            w1rep = bf[:, N + 2 * D : N + 3 * D]
            emyT = bf[:, _CMY:]
            ones_row = rowsS[:, :]

            # ---- prologue -------------------------------------------
            # er_myT = e_myT + r (bf16), left via PE, B = clip((L-lo)/h)
            er_myT = const.tile([D, IPC], bf16)
            nc.vector.tensor_scalar(er_myT[:], emyT, rc, None, A_OP.add)
            left_ps = ps.tile([D, IPC], f32, tag="ps")
            nc.tensor.matmul(left_ps[:], wlT, er_myT[:], start=True, stop=True)
            B1 = const.tile([D, IPC], f32)
            nc.vector.tensor_scalar(
                B1[:], left_ps[:], 1.0 / HSTEP, -LO / HSTEP, A_OP.mult, A_OP.add
            )
            B = const.tile([D, IPC], f32)
            nc.vector.tensor_scalar(B[:], B1[:], 0.0, Q - 1.0, A_OP.max, A_OP.min)
            corr_t = const.tile([D, IPC], bf16)
            nc.vector.tensor_scalar(
                corr_t[:], B1[:], Q - 1.0, 0.0, A_OP.subtract, A_OP.max
            )
            # bias accumulation: 0.1*lin_i + corr_i  (then + b1)
            lini_ps = ps.tile([IPC, 1], f32, tag="ps")
            nc.tensor.matmul(lini_ps[:], er_myT[:], u01b, start=True, stop=False)
            nc.tensor.matmul(lini_ps[:], corr_t[:], ch9b, start=False, stop=True)
            biasvec = const.tile([IPC, 1], f32)
            nc.vector.tensor_scalar(biasvec[:], lini_ps[:], b1f, None, A_OP.add)

            # rbT = bf16(Wr @ e_allT + b0)
            rbT = big.tile([D, N], bf16)
            for hh in range(2):
                sl = slice(hh * H, (hh + 1) * H)
                rp_ps = ps.tile([D, H], f32, tag="ps")
                nc.tensor.matmul(rp_ps[:], wrT, eallT[:, sl], start=True, stop=True)
                nc.vector.tensor_scalar(rbT[:, sl], rp_ps[:], b0c, None, A_OP.add)

            # ---- main loop over interpolation levels ----------------
            # t1 = |B - q| for all q upfront on ACT so DVE never stalls on
            # the cross-engine dependency; G tiles split ACT/DVE; the
            # stop-q (15) G is produced early so the last accumulation has
            # no wait.
            z0 = psz.tile([D, H], f32, tag="z")
            z1 = psz.tile([D, H], f32, tag="z")

            t1s = []
            for q in range(Q):
                t1 = hold.tile([D, IPC], bf16, tag=f"t1_{q}")
                nc.scalar.activation(t1[:], B[:], AF.Abs, bias=nqc(q), scale=1.0)
                t1s.append(t1)

            # ---- c-branch: c_fs over all j --------------------------
            cl_ps = ps.tile([D, 1], f32, tag="ps")
            nc.tensor.matmul(cl_ps[:], wlT, cb, start=True, stop=True)
            cl = const.tile([D, 1], f32)
            nc.scalar.copy(cl[:], cl_ps[:])
            clb = const.tile([D, 1], bf16)
            nc.vector.tensor_copy(clb[:], cl_ps[:])
            clw_ps = ps.tile([1, 1], f32, tag="ps")
            nc.tensor.matmul(clw_ps[:], clb[:], w1b, start=True, stop=True)
            bc = const.tile([1, 1], f32)
            nc.vector.tensor_scalar(bc[:], clw_ps[:], 0.1, b1f, A_OP.mult, A_OP.add)

            cfs_row = const.tile([1, N], f32)
            Ac = big.tile([D, N], bf16)
            nc.scalar.activation(Ac[:], rbT[:], AF.Relu, bias=cl[:], scale=1.0)
            crep = []
            for hh in range(2):
                sl = slice(hh * H, (hh + 1) * H)
                zc_ps = ps.tile([1, H], f32, tag="ps")
                nc.tensor.matmul(zc_ps[:], w9b, Ac[:, sl], start=True, stop=False)
                nc.tensor.matmul(zc_ps[:], w01b, rbT[:, sl], start=False, stop=True)
                nc.scalar.activation(
                    cfs_row[:, sl], zc_ps[:], AF.Sigmoid, bias=bc[:], scale=1.0
                )
                cr = psc.tile([D, H], f32, tag="crep")
                nc.tensor.matmul(
                    cr[:], ones_row, cfs_row[0:1, sl], start=True, stop=True
                )
                crep.append(cr)


            GACT = set(range(14 - N_GACT, 14))        # ACT-built G levels
            gdve_order = [0, Q - 1] + [q for q in range(1, Q - 1) if q not in GACT]
            Gs = {}
            for q in GACT:
                G = hold.tile([D, N], bf16, tag=f"G_{q}")
                nc.scalar.activation(G[:], rbT[:], AF.Relu, bias=lqc(q), scale=1.0)
                Gs[q] = G

            def emit_sq(q):
                t2 = sw.tile([D, IPC], bf16, tag="t2")
                nc.vector.tensor_scalar(
                    t2[:], t1s[q][:], 1.0, 0.0, A_OP.subtract, A_OP.min
                )
                Sq = hold.tile([D, IPC], bf16, tag=f"Sq_{q}")
                nc.vector.tensor_scalar(Sq[:], t2[:], w9nc, None, A_OP.mult)
                return Sq

            def emit_gdve(q):
                G = hold.tile([D, N], bf16, tag=f"G_{q}")
                lq = LO + q * HSTEP
                nc.vector.tensor_scalar(
                    G[:], rbT[:], float(lq), 0.0, A_OP.add, A_OP.max
                )
                Gs[q] = G

            Sqs = {}
            gd = iter(gdve_order)
            for q in range(Q):
                Sqs[q] = emit_sq(q)
                nq = next(gd, None)
                if nq is not None:
                    emit_gdve(nq)

            for q in range(Q):
                st = q == 0
                sp = q == Q - 1
                nc.tensor.matmul(z0[:], Sqs[q][:], Gs[q][:, :H], start=st, stop=sp)
                nc.tensor.matmul(z1[:], Sqs[q][:], Gs[q][:, H:], start=st, stop=sp)
                if q == 0:
                    # fold 0.1*lin_j into every row (PSUM accumulation is
                    # order-independent)
                    nc.tensor.matmul(z0[:], w1rep, rbT[:, :H], start=False, stop=False)
                    nc.tensor.matmul(z1[:], w1rep, rbT[:, H:], start=False, stop=False)

            # ---- epilogue -------------------------------------------
            # per half: sigmoid (ACT) then fused multiply+max-reduce (DVE)
            rfs = big.tile([D, N], f32)
            prod = big.tile([D, N], f32)
            outc2 = const.tile([IPC, 2], f32)
            for hh, zb in ((0, z0), (1, z1)):
                sl = slice(hh * H, (hh + 1) * H)
                nc.scalar.activation(
                    rfs[:, sl], zb[:], AF.Sigmoid, bias=biasvec[:], scale=1.0
                )
                nc.vector.tensor_tensor(
                    prod[:, sl], rfs[:, sl], crep[hh][:], A_OP.mult
                )
                nc.vector.tensor_reduce(
                    outc2[:, hh : hh + 1],
                    prod[:, sl],
                    axis=mybir.AxisListType.X,
                    op=A_OP.max,
                )
            outc = const.tile([IPC, 1], f32)
            nc.vector.tensor_tensor(
                outc[:], outc2[:, 0:1], outc2[:, 1:2], A_OP.max
            )
            # transpose [IPC, 1] -> [1, IPC] so the output DMA is one line
            orow_ps = ps.tile([1, IPC], f32, tag="ps")
            nc.tensor.matmul(orow_ps[:], outc[:], ident[:], start=True, stop=True)
            orow = const.tile([1, IPC], f32)
            nc.scalar.copy(orow[:], orow_ps[:])
            nc.sync.dma_start(d_out[:], orow[:])

    return nc


def _host_prep(anon_e_emb, e_table, c_emb, r_emb, W0, b0, W1, b1):
    f = np.float32
    bft = ml_dtypes.bfloat16
    anon_e_emb = np.asarray(anon_e_emb, f)
    e_table = np.asarray(e_table, f)
    c_emb = np.asarray(c_emb, f)
    r_emb = np.asarray(r_emb, f)
    W0 = np.asarray(W0, f)
    b0 = np.asarray(b0, f)
    W1 = np.asarray(W1, f)
    b1 = np.asarray(b1, f)

    Wl = W0[:, :D]
    e_all = np.concatenate([e_table, anon_e_emb], axis=0)  # [N, D]
    e_allT = np.ascontiguousarray(e_all.T)  # [D, N]

    bf_base = np.zeros((D, _BF_COLS), bft)
    bf_base[:, :N] = e_allT.astype(bft)
    bf_base[:, N : N + D] = Wl.T.astype(bft)
    bf_base[:, N + D : N + 2 * D] = W0[:, D:].T.astype(bft)
    bf_base[:, N + 2 * D : N + 3 * D] = np.tile(
        (0.1 * W1).astype(bft)[:, None], (1, D)
    )
    bf_base[:, _C0 + 0] = r_emb.astype(bft)
    bf_base[:, _C0 + 1] = b0.astype(bft)
    bf_base[:, _C0 + 2] = (-0.9 * W1).astype(bft)
    bf_base[:, _C0 + 3] = (0.1 * (W1 @ Wl)).astype(bft)
    bf_base[:, _C0 + 4] = (HSTEP * 0.9 * W1).astype(bft)
    bf_base[:, _C0 + 5] = c_emb.astype(bft)
    bf_base[:, _C0 + 6] = W1.astype(bft)
    bf_base[:, _C0 + 7] = (0.9 * W1).astype(bft)
    bf_base[:, _C0 + 8] = (0.1 * W1).astype(bft)
    for q in range(Q):
        bf_base[:, _CLQ + q] = np.float32(LO + q * HSTEP).astype(bft)
        bf_base[:, _CNQ + q] = np.float32(-q).astype(bft)

    rows = np.ones((1, D), f)
    ident = np.eye(D, dtype=f)
    b1f = float(b1[0])

    in_maps = []
    for c in range(NCORES):
        bf_pack = bf_base.copy()
        bf_pack[:, _CMY:] = e_allT[:, c * IPC : (c + 1) * IPC].astype(bft)
        in_maps.append({"bf_pack": bf_pack, "rows": rows, "ident": ident})
    return in_maps, b1f


def _install_ntff_shim():
    """Provide antenv.axon_hooks (missing in this image) so that
    run_bass_kernel_spmd(trace=True) can collect NTFF profiles."""
    import sys
    import types

    if "antenv.axon_hooks" in sys.modules:
        return
    try:
        import antenv
        from trn_agent_boot.trn_boot import _ntff_profile_via_ctypes
    except ImportError:
        return
    mod = types.ModuleType("antenv.axon_hooks")
    state = {"hook": None}
    mod.set_axon_ntff_profile_hook = lambda h: state.__setitem__("hook", h)
    mod.get_axon_ntff_profile_hook = lambda: state["hook"]
    sys.modules["antenv.axon_hooks"] = mod
    antenv.axon_hooks = mod
    try:
        mod.set_axon_ntff_profile_hook(
            _ntff_profile_via_ctypes("/opt/axon/libaxon_pjrt.so")
        )
    except Exception:
        pass


def kernel_ex(inputs: dict, trace: bool = False):
    """Run on 8 NeuronCores; returns (out [N] float32, BassKernelResults)."""
    from concourse.bass_utils import run_bass_kernel_spmd

    if trace:
        _install_ntff_shim()

    in_maps, b1f = _host_prep(**inputs)
    key = (round(b1f, 10),)
    nc = _PROGRAM_CACHE.get(key)
    if nc is None:
        nc = _build_program(b1f)
        nc.finalize()
        _PROGRAM_CACHE[key] = nc

    res = run_bass_kernel_spmd(
        nc, in_maps, core_ids=list(range(NCORES)), trace=trace
    )
    out = np.concatenate(
        [
            np.asarray(res.results[c]["out"], np.float32).reshape(IPC)
            for c in range(NCORES)
        ]
    )
    return out, res


def kernel(**inputs) -> np.ndarray:
    out, _ = kernel_ex(inputs, trace=False)
    return out


# revision 32
# speedup vs baseline: 1.2410x; 1.0303x over previous
"""FALCON ObjectSomeValuesFrom forward kernel for Trainium2 (8 NeuronCores).

Math (reference):
    e_all = concat(e_table, anon_e_emb)            # [n, d], n=1024, d=128
    Wl, Wr = W0[:, :d], W0[:, d:]
    c_fs  = sigmoid(leaky(c@Wl.T + e_all@Wr.T + b0) @ W1 + b1)        # [n]
    left  = (e_all + r) @ Wl.T ; rp = e_all @ Wr.T + b0
    z_ij  = leaky(left_i + rp_j) @ W1                                  # [n, n]
    out_i = max_j sigmoid(z_ij + b1) * c_fs[j]

Algorithm (quantized interpolation): with leaky(x) = 0.1 x + 0.9 relu(x),
    z_ij = 0.1 (lin_i + lin_j) + sum_k w9_k relu(L_ik + rp_jk),  w9 = 0.9 W1.
Clip L to [-A, A] with A >= max|rp| and correct exactly:
    relu(L + rp) = relu(clip(L) + rp) + relu(L - A)    (the last term is
    j-independent -> folded into the per-i sigmoid bias).
Quantize clip(L) on a Q-level grid l_q, piecewise-LINEAR interp in L:
    relu(L + rp) ~= sum_q hat((L - lo)/h - q) * relu(l_q + rp)
so the [n, n, d] relu tensor is replaced by Q relu tables G_q = relu(rp + l_q)
([d, n] each, built by DVE/ACT) and interpolation weights
S_q[k, i] = w9_k hat(B_ik - q) ([d, n_i]; ACT Abs + 2 cheap DVE ops per q),
contracted on the PE: z_relu = sum_q S_q^T @ G_q (PSUM accumulation over q,
4-strip concurrent matmuls).  Elementwise work drops from n_i to Q tiles.

The output column [IPC, 1] is PE-transposed to a [1, IPC] row before the
store DMA (a partition-strided 4 B/line DMA costs ~8 us; one 512 B line is
cheap).

Sharding: i-rows split across 8 cores; e_table/weights/embeddings replicated;
final max over j is local per core.
"""

import numpy as np
import ml_dtypes

N = 1024
D = 128
NCORES = 8
IPC = N // NCORES  # i rows per core = 128
H = 512            # PSUM bank free size (fp32)

Q = 16             # interpolation levels
A_CLIP = 0.5       # L clip range; exact correction term added for L > A
LO = -A_CLIP
HSTEP = 2.0 * A_CLIP / (Q - 1)

N_GACT = 3         # G tiles built by ACT (rest on DVE)

_PROGRAM_CACHE: dict = {}

# bf16 pack layout (columns):
#   e_allT[1024] | wlT[128] | wrT[128] | w1rep01[128] |
#   rc | b0 | w9n | u01 | ch9 | c | w1 | w9 | w01 | lq[Q] | negq[Q] |
#   e_myT[128]
_C0 = N + 3 * D
_NSCAL = 9
_CLQ = _C0 + _NSCAL
_CNQ = _CLQ + Q
_CMY = _CNQ + Q
_BF_COLS = _CMY + D


def _build_program(b1f: float):
    import concourse.bacc as bacc
    import concourse.mybir as mybir
    import concourse.tile as tile

    f32 = mybir.dt.float32
    bf16 = mybir.dt.bfloat16
    A_OP = mybir.AluOpType
    AF = mybir.ActivationFunctionType

    nc = bacc.Bacc(None, target_bir_lowering=False, name="falcon_fwd")

    d_bf = nc.dram_tensor("bf_pack", [D, _BF_COLS], bf16, kind="ExternalInput")
    d_rows = nc.dram_tensor("rows", [1, D], bf16, kind="ExternalInput")
    d_ident = nc.dram_tensor("ident", [D, D], f32, kind="ExternalInput")
    d_out = nc.dram_tensor("out", [1, IPC], f32, kind="ExternalOutput")

    with tile.TileContext(nc) as tc:
        with (
            tc.tile_pool(name="const", bufs=1) as const,
            tc.tile_pool(name="big", bufs=1) as big,
            tc.tile_pool(name="sw", bufs=4) as sw,
            tc.tile_pool(name="hold", bufs=1) as hold,
            tc.tile_pool(name="ps", bufs=3, space="PSUM") as ps,
            tc.tile_pool(name="psz", bufs=2, space="PSUM") as psz,
            tc.tile_pool(name="psc", bufs=2, space="PSUM") as psc,
        ):
            # ---- input DMAs: weights/cols tail first, then e_allT -----
            bf = big.tile([D, _BF_COLS], bf16)
            rows_raw = const.tile([1, D], bf16)
            ident = const.tile([D, D], f32)
            nc.sync.dma_start(bf[:, N:], d_bf[:, N:])  # weights + cols + e_myT
            nc.sync.dma_start(bf[:, :H], d_bf[:, :H])
            nc.sync.dma_start(bf[:, H:N], d_bf[:, H:N])
            nc.sync.dma_start(rows_raw[:], d_rows[:])
            nc.sync.dma_start(ident[:], d_ident[:])

            # dummy sigmoid first: forces the one ACT table set that
            # contains {sigmoid, abs, relu, copy} to load exactly once
            dum = const.tile([1, 1], f32)
            nc.vector.memset(dum[:], 0.0)
            nc.scalar.activation(dum[:], dum[:], AF.Sigmoid)

            # funnel the scalar-pointer columns through one DVE copy
            # (bf16 in the DMA pack for wide lines; f32 on-chip for ptr ops)
            colsB = const.tile([D, _NSCAL + 2 * Q], f32)
            nc.vector.tensor_copy(colsB[:], bf[:, _C0:_CMY])
            rowsS = const.tile([1, D], bf16)
            nc.vector.tensor_copy(rowsS[:], rows_raw[:])

            rc = colsB[:, 0:1]
            b0c = colsB[:, 1:2]
            w9nc = colsB[:, 2:3]
            # matmul rhs columns must be bf16 -> slice the DMA'd pack
            u01b = bf[:, _C0 + 3 : _C0 + 4]
            ch9b = bf[:, _C0 + 4 : _C0 + 5]
            cb = bf[:, _C0 + 5 : _C0 + 6]
            w1b = bf[:, _C0 + 6 : _C0 + 7]
            w9b = bf[:, _C0 + 7 : _C0 + 8]
            w01b = bf[:, _C0 + 8 : _C0 + 9]
            lqc = lambda q: colsB[:, _NSCAL + q : _NSCAL + q + 1]
            nqc = lambda q: colsB[:, _NSCAL + Q + q : _NSCAL + Q + q + 1]
            eallT = bf[:, :N]
            wlT = bf[:, N : N + D]
            wrT = bf[:, N + D : N + 2 * D]
            w1rep = bf[:, N + 2 * D : N + 3 * D]
            emyT = bf[:, _CMY:]
            ones_row = rowsS[:, :]

            # ---- prologue -------------------------------------------
            # er_myT = e_myT + r (bf16), left via PE, B = clip((L-lo)/h)
            er_myT = const.tile([D, IPC], bf16)
            nc.vector.tensor_scalar(er_myT[:], emyT, rc, None, A_OP.add)
            left_ps = ps.tile([D, IPC], f32, tag="ps")
            nc.tensor.matmul(left_ps[:], wlT, er_myT[:], start=True, stop=True)
            B1 = const.tile([D, IPC], f32)
            nc.vector.tensor_scalar(
                B1[:], left_ps[:], 1.0 / HSTEP, -LO / HSTEP, A_OP.mult, A_OP.add
            )
            B = const.tile([D, IPC], f32)
            nc.vector.tensor_scalar(B[:], B1[:], 0.0, Q - 1.0, A_OP.max, A_OP.min)
            corr_t = const.tile([D, IPC], bf16)
            nc.vector.tensor_scalar(
                corr_t[:], B1[:], Q - 1.0, 0.0, A_OP.subtract, A_OP.max
            )
            # bias accumulation: 0.1*lin_i + corr_i  (then + b1)
            lini_ps = ps.tile([IPC, 1], f32, tag="ps")
            nc.tensor.matmul(lini_ps[:], er_myT[:], u01b, start=True, stop=False)
            nc.tensor.matmul(lini_ps[:], corr_t[:], ch9b, start=False, stop=True)
            biasvec = const.tile([IPC, 1], f32)
            nc.vector.tensor_scalar(biasvec[:], lini_ps[:], b1f, None, A_OP.add)

            # rbT = bf16(Wr @ e_allT + b0)
            rbT = big.tile([D, N], bf16)
            for hh in range(2):
                sl = slice(hh * H, (hh + 1) * H)
                rp_ps = ps.tile([D, H], f32, tag="ps")
                nc.tensor.matmul(rp_ps[:], wrT, eallT[:, sl], start=True, stop=True)
                nc.vector.tensor_scalar(rbT[:, sl], rp_ps[:], b0c, None, A_OP.add)

            # ---- main loop over interpolation levels ----------------
            # t1 = |B - q| for all q upfront on ACT so DVE never stalls on
            # the cross-engine dependency; G tiles split ACT/DVE; the
            # stop-q (15) G is produced early so the last accumulation has
            # no wait.
            z0 = psz.tile([D, H], f32, tag="z")
            z1 = psz.tile([D, H], f32, tag="z")

            t1s = []
            for q in range(Q):
                t1 = hold.tile([D, IPC], bf16, tag=f"t1_{q}")
                nc.scalar.activation(t1[:], B[:], AF.Abs, bias=nqc(q), scale=1.0)
                t1s.append(t1)

            # ---- c-branch: c_fs over all j --------------------------
            cl_ps = ps.tile([D, 1], f32, tag="ps")
            nc.tensor.matmul(cl_ps[:], wlT, cb, start=True, stop=True)
            cl = const.tile([D, 1], f32)
            nc.scalar.copy(cl[:], cl_ps[:])
            clb = const.tile([D, 1], bf16)
            nc.vector.tensor_copy(clb[:], cl_ps[:])
            clw_ps = ps.tile([1, 1], f32, tag="ps")
            nc.tensor.matmul(clw_ps[:], clb[:], w1b, start=True, stop=True)
            bc = const.tile([1, 1], f32)
            nc.vector.tensor_scalar(bc[:], clw_ps[:], 0.1, b1f, A_OP.mult, A_OP.add)

            cfs_row = const.tile([1, N], bf16)
            Ac = big.tile([D, N], bf16)
            nc.scalar.activation(Ac[:], rbT[:], AF.Relu, bias=cl[:], scale=1.0)
            for hh in range(2):
                sl = slice(hh * H, (hh + 1) * H)
                zc_ps = ps.tile([1, H], f32, tag="ps")
                nc.tensor.matmul(zc_ps[:], w9b, Ac[:, sl], start=True, stop=False)
                nc.tensor.matmul(zc_ps[:], w01b, rbT[:, sl], start=False, stop=True)
                nc.scalar.activation(
                    cfs_row[:, sl], zc_ps[:], AF.Sigmoid, bias=bc[:], scale=1.0
                )


            GACT = set(range(14 - N_GACT, 14))        # ACT-built G levels
            gdve_order = [0, Q - 1] + [q for q in range(1, Q - 1) if q not in GACT]
            Gs = {}
            for q in GACT:
                G = hold.tile([D, N], bf16, tag=f"G_{q}")
                nc.scalar.activation(G[:], rbT[:], AF.Relu, bias=lqc(q), scale=1.0)
                Gs[q] = G

            def emit_sq(q):
                t2 = sw.tile([D, IPC], bf16, tag="t2")
                nc.vector.tensor_scalar(
                    t2[:], t1s[q][:], 1.0, 0.0, A_OP.subtract, A_OP.min
                )
                Sq = hold.tile([D, IPC], bf16, tag=f"Sq_{q}")
                nc.vector.tensor_scalar(Sq[:], t2[:], w9nc, None, A_OP.mult)
                return Sq

            def emit_gdve(q):
                G = hold.tile([D, N], bf16, tag=f"G_{q}")
                lq = LO + q * HSTEP
                nc.vector.tensor_scalar(
                    G[:], rbT[:], float(lq), 0.0, A_OP.add, A_OP.max
                )
                Gs[q] = G

            Sqs = {}
            gd = iter(gdve_order)
            for q in range(Q):
                Sqs[q] = emit_sq(q)
                nq = next(gd, None)
                if nq is not None:
                    emit_gdve(nq)

            for q in range(Q):
                st = q == 0
                sp = q == Q - 1
                nc.tensor.matmul(z0[:], Sqs[q][:], Gs[q][:, :H], start=st, stop=sp)
                nc.tensor.matmul(z1[:], Sqs[q][:], Gs[q][:, H:], start=st, stop=sp)
                if q == 0:
                    # fold 0.1*lin_j into every row (PSUM accumulation is
                    # order-independent)
                    nc.tensor.matmul(z0[:], w1rep, rbT[:, :H], start=False, stop=False)
                    nc.tensor.matmul(z1[:], w1rep, rbT[:, H:], start=False, stop=False)

            # crep broadcast AFTER the loop matmuls so the in-order PE
            # queue never stalls the z accumulation on the cfs sigmoids
            crep = []
            for hh in range(2):
                sl = slice(hh * H, (hh + 1) * H)
                cr = psc.tile([D, H], f32, tag="crep")
                nc.tensor.matmul(
                    cr[:], ones_row, cfs_row[0:1, sl], start=True, stop=True
                )
                crep.append(cr)

            # ---- epilogue (quarter-pipelined) ------------------------
            QH = H // 2
            rfs = big.tile([D, N], f32)
            prod = big.tile([D, N], f32)
            outc4 = const.tile([IPC, 4], f32)
            for qq in range(4):
                hh = qq // 2
                zb = (z0, z1)[hh]
                zsl = slice((qq % 2) * QH, (qq % 2) * QH + QH)
                sl = slice(qq * QH, (qq + 1) * QH)
                nc.scalar.activation(
                    rfs[:, sl], zb[:, zsl], AF.Sigmoid, bias=biasvec[:], scale=1.0
                )
                nc.vector.tensor_tensor(
                    prod[:, sl], rfs[:, sl], crep[hh][:, zsl], A_OP.mult
                )
                nc.vector.tensor_reduce(
                    outc4[:, qq : qq + 1],
                    prod[:, sl],
                    axis=mybir.AxisListType.X,
                    op=A_OP.max,
                )
            outc2 = const.tile([IPC, 2], f32)
            nc.vector.tensor_tensor(
                outc2[:], outc4[:, 0:2], outc4[:, 2:4], A_OP.max
            )
            outc = const.tile([IPC, 1], f32)
            nc.vector.tensor_tensor(
                outc[:], outc2[:, 0:1], outc2[:, 1:2], A_OP.max
            )
            # transpose [IPC, 1] -> [1, IPC] so the output DMA is one line
            orow_ps = ps.tile([1, IPC], f32, tag="ps")
            nc.tensor.matmul(orow_ps[:], outc[:], ident[:], start=True, stop=True)
            orow = const.tile([1, IPC], f32)
            nc.scalar.copy(orow[:], orow_ps[:])
            nc.sync.dma_start(d_out[:], orow[:])

    return nc


def _host_prep(anon_e_emb, e_table, c_emb, r_emb, W0, b0, W1, b1):
    f = np.float32
    bft = ml_dtypes.bfloat16
    anon_e_emb = np.asarray(anon_e_emb, f)
    e_table = np.asarray(e_table, f)
    c_emb = np.asarray(c_emb, f)
    r_emb = np.asarray(r_emb, f)
    W0 = np.asarray(W0, f)
    b0 = np.asarray(b0, f)
    W1 = np.asarray(W1, f)
    b1 = np.asarray(b1, f)

    Wl = W0[:, :D]
    e_all = np.concatenate([e_table, anon_e_emb], axis=0)  # [N, D]
    e_allT = np.ascontiguousarray(e_all.T)  # [D, N]

    bf_base = np.zeros((D, _BF_COLS), bft)
    bf_base[:, :N] = e_allT.astype(bft)
    bf_base[:, N : N + D] = Wl.T.astype(bft)
    bf_base[:, N + D : N + 2 * D] = W0[:, D:].T.astype(bft)
    bf_base[:, N + 2 * D : N + 3 * D] = np.tile(
        (0.1 * W1).astype(bft)[:, None], (1, D)
    )
    bf_base[:, _C0 + 0] = r_emb.astype(bft)
    bf_base[:, _C0 + 1] = b0.astype(bft)
    bf_base[:, _C0 + 2] = (-0.9 * W1).astype(bft)
    bf_base[:, _C0 + 3] = (0.1 * (W1 @ Wl)).astype(bft)
    bf_base[:, _C0 + 4] = (HSTEP * 0.9 * W1).astype(bft)
    bf_base[:, _C0 + 5] = c_emb.astype(bft)
    bf_base[:, _C0 + 6] = W1.astype(bft)
    bf_base[:, _C0 + 7] = (0.9 * W1).astype(bft)
    bf_base[:, _C0 + 8] = (0.1 * W1).astype(bft)
    for q in range(Q):
        bf_base[:, _CLQ + q] = np.float32(LO + q * HSTEP).astype(bft)
        bf_base[:, _CNQ + q] = np.float32(-q).astype(bft)

    rows = np.ones((1, D), bft)
    ident = np.eye(D, dtype=f)
    b1f = float(b1[0])

    in_maps = []
    for c in range(NCORES):
        bf_pack = bf_base.copy()
        bf_pack[:, _CMY:] = e_allT[:, c * IPC : (c + 1) * IPC].astype(bft)
        in_maps.append({"bf_pack": bf_pack, "rows": rows, "ident": ident})
    return in_maps, b1f


def _install_ntff_shim():
    """Provide antenv.axon_hooks (missing in this image) so that
    run_bass_kernel_spmd(trace=True) can collect NTFF profiles."""
    import sys
    import types

    if "antenv.axon_hooks" in sys.modules:
        return
    try:
        import antenv
        from trn_agent_boot.trn_boot import _ntff_profile_via_ctypes
    except ImportError:
        return
    mod = types.ModuleType("antenv.axon_hooks")
    state = {"hook": None}
    mod.set_axon_ntff_profile_hook = lambda h: state.__setitem__("hook", h)
    mod.get_axon_ntff_profile_hook = lambda: state["hook"]
    sys.modules["antenv.axon_hooks"] = mod
    antenv.axon_hooks = mod
    try:
        mod.set_axon_ntff_profile_hook(
            _ntff_profile_via_ctypes("/opt/axon/libaxon_pjrt.so")
        )
    except Exception:
        pass


def kernel_ex(inputs: dict, trace: bool = False):
    """Run on 8 NeuronCores; returns (out [N] float32, BassKernelResults)."""
    from concourse.bass_utils import run_bass_kernel_spmd

    if trace:
        _install_ntff_shim()

    in_maps, b1f = _host_prep(**inputs)
    key = (round(b1f, 10),)
    nc = _PROGRAM_CACHE.get(key)
    if nc is None:
        nc = _build_program(b1f)
        nc.finalize()
        _PROGRAM_CACHE[key] = nc

    res = run_bass_kernel_spmd(
        nc, in_maps, core_ids=list(range(NCORES)), trace=trace
    )
    out = np.concatenate(
        [
            np.asarray(res.results[c]["out"], np.float32).reshape(IPC)
            for c in range(NCORES)
        ]
    )
    return out, res


def kernel(**inputs) -> np.ndarray:
    out, _ = kernel_ex(inputs, trace=False)
    return out


# revision 34
# speedup vs baseline: 1.2995x; 1.0472x over previous
"""FALCON ObjectSomeValuesFrom forward kernel for Trainium2 (8 NeuronCores).

Math (reference):
    e_all = concat(e_table, anon_e_emb)            # [n, d], n=1024, d=128
    Wl, Wr = W0[:, :d], W0[:, d:]
    c_fs  = sigmoid(leaky(c@Wl.T + e_all@Wr.T + b0) @ W1 + b1)        # [n]
    left  = (e_all + r) @ Wl.T ; rp = e_all @ Wr.T + b0
    z_ij  = leaky(left_i + rp_j) @ W1                                  # [n, n]
    out_i = max_j sigmoid(z_ij + b1) * c_fs[j]

Algorithm (quantized interpolation): with leaky(x) = 0.1 x + 0.9 relu(x),
    z_ij = 0.1 (lin_i + lin_j) + sum_k w9_k relu(L_ik + rp_jk),  w9 = 0.9 W1.
Clip L to [-A, A] with A >= max|rp| and correct exactly:
    relu(L + rp) = relu(clip(L) + rp) + relu(L - A)    (the last term is
    j-independent -> folded into the per-i sigmoid bias).
Quantize clip(L) on a Q-level grid l_q, piecewise-LINEAR interp in L:
    relu(L + rp) ~= sum_q hat((L - lo)/h - q) * relu(l_q + rp)
so the [n, n, d] relu tensor is replaced by Q relu tables G_q = relu(rp + l_q)
([d, n] each, built by DVE/ACT) and interpolation weights
S_q[k, i] = w9_k hat(B_ik - q) ([d, n_i]; ACT Abs + 2 cheap DVE ops per q),
contracted on the PE: z_relu = sum_q S_q^T @ G_q (PSUM accumulation over q,
4-strip concurrent matmuls).  Elementwise work drops from n_i to Q tiles.

The output column [IPC, 1] is PE-transposed to a [1, IPC] row before the
store DMA (a partition-strided 4 B/line DMA costs ~8 us; one 512 B line is
cheap).

Sharding: i-rows split across 8 cores; e_table/weights/embeddings replicated;
final max over j is local per core.
"""

import numpy as np
import ml_dtypes

N = 1024
D = 128
NCORES = 8
IPC = N // NCORES  # i rows per core = 128
H = 512            # PSUM bank free size (fp32)

Q = 12             # interpolation levels
A_CLIP = 0.5       # L clip range; exact correction term added for L > A
LO = -A_CLIP
HSTEP = 2.0 * A_CLIP / (Q - 1)

N_GACT = 3         # G tiles built by ACT (rest on DVE)

_PROGRAM_CACHE: dict = {}

# bf16 pack layout (columns):
#   e_allT[1024] | wlT[128] | wrT[128] | w1rep01[128] |
#   rc | b0 | w9n | u01 | ch9 | c | w1 | w9 | w01 | lq[Q] | negq[Q] |
#   e_myT[128]
_C0 = N + 3 * D
_NSCAL = 9
_CLQ = _C0 + _NSCAL
_CNQ = _CLQ + Q
_CMY = _CNQ + Q
_BF_COLS = _CMY + D


def _build_program(b1f: float):
    import concourse.bacc as bacc
    import concourse.mybir as mybir
    import concourse.tile as tile

    f32 = mybir.dt.float32
    bf16 = mybir.dt.bfloat16
    A_OP = mybir.AluOpType
    AF = mybir.ActivationFunctionType

    nc = bacc.Bacc(None, target_bir_lowering=False, name="falcon_fwd")

    d_bf = nc.dram_tensor("bf_pack", [D, _BF_COLS], bf16, kind="ExternalInput")
    d_rows = nc.dram_tensor("rows", [1, D], bf16, kind="ExternalInput")
    d_ident = nc.dram_tensor("ident", [D, D], f32, kind="ExternalInput")
    d_out = nc.dram_tensor("out", [1, IPC], f32, kind="ExternalOutput")

    with tile.TileContext(nc) as tc:
        with (
            tc.tile_pool(name="const", bufs=1) as const,
            tc.tile_pool(name="big", bufs=1) as big,
            tc.tile_pool(name="sw", bufs=4) as sw,
            tc.tile_pool(name="hold", bufs=1) as hold,
            tc.tile_pool(name="ps", bufs=3, space="PSUM") as ps,
            tc.tile_pool(name="psz", bufs=2, space="PSUM") as psz,
            tc.tile_pool(name="psc", bufs=2, space="PSUM") as psc,
        ):
            # ---- input DMAs: weights/cols tail first, then e_allT -----
            bf = big.tile([D, _BF_COLS], bf16)
            rows_raw = const.tile([1, D], bf16)
            ident = const.tile([D, D], f32)
            nc.sync.dma_start(bf[:, N:], d_bf[:, N:])  # weights + cols + e_myT
            nc.sync.dma_start(bf[:, :H], d_bf[:, :H])
            nc.sync.dma_start(bf[:, H:N], d_bf[:, H:N])
            nc.sync.dma_start(rows_raw[:], d_rows[:])
            nc.sync.dma_start(ident[:], d_ident[:])

            # dummy sigmoid first: forces the one ACT table set that
            # contains {sigmoid, abs, relu, copy} to load exactly once
            dum = const.tile([1, 1], f32)
            nc.vector.memset(dum[:], 0.0)
            nc.scalar.activation(dum[:], dum[:], AF.Sigmoid)

            # funnel the scalar-pointer columns through one DVE copy
            # (bf16 in the DMA pack for wide lines; f32 on-chip for ptr ops)
            colsB = const.tile([D, _NSCAL + 2 * Q], f32)
            nc.vector.tensor_copy(colsB[:], bf[:, _C0:_CMY])
            rowsS = const.tile([1, D], bf16)
            nc.vector.tensor_copy(rowsS[:], rows_raw[:])

            rc = colsB[:, 0:1]
            b0c = colsB[:, 1:2]
            w9nc = colsB[:, 2:3]
            # matmul rhs columns must be bf16 -> slice the DMA'd pack
            u01b = bf[:, _C0 + 3 : _C0 + 4]
            ch9b = bf[:, _C0 + 4 : _C0 + 5]
            cb = bf[:, _C0 + 5 : _C0 + 6]
            w1b = bf[:, _C0 + 6 : _C0 + 7]
            w9b = bf[:, _C0 + 7 : _C0 + 8]
            w01b = bf[:, _C0 + 8 : _C0 + 9]
            lqc = lambda q: colsB[:, _NSCAL + q : _NSCAL + q + 1]
            nqc = lambda q: colsB[:, _NSCAL + Q + q : _NSCAL + Q + q + 1]
            eallT = bf[:, :N]
            wlT = bf[:, N : N + D]
            wrT = bf[:, N + D : N + 2 * D]
            w1rep = bf[:, N + 2 * D : N + 3 * D]
            emyT = bf[:, _CMY:]
            ones_row = rowsS[:, :]

            # ---- prologue -------------------------------------------
            # er_myT = e_myT + r (bf16), left via PE, B = clip((L-lo)/h)
            er_myT = const.tile([D, IPC], bf16)
            nc.vector.tensor_scalar(er_myT[:], emyT, rc, None, A_OP.add)
            left_ps = ps.tile([D, IPC], f32, tag="ps")
            nc.tensor.matmul(left_ps[:], wlT, er_myT[:], start=True, stop=True)
            B1 = const.tile([D, IPC], f32)
            nc.vector.tensor_scalar(
                B1[:], left_ps[:], 1.0 / HSTEP, -LO / HSTEP, A_OP.mult, A_OP.add
            )
            B = const.tile([D, IPC], f32)
            nc.vector.tensor_scalar(B[:], B1[:], 0.0, Q - 1.0, A_OP.max, A_OP.min)
            corr_t = const.tile([D, IPC], bf16)
            nc.vector.tensor_scalar(
                corr_t[:], B1[:], Q - 1.0, 0.0, A_OP.subtract, A_OP.max
            )
            # bias accumulation: 0.1*lin_i + corr_i  (then + b1)
            lini_ps = ps.tile([IPC, 1], f32, tag="ps")
            nc.tensor.matmul(lini_ps[:], er_myT[:], u01b, start=True, stop=False)
            nc.tensor.matmul(lini_ps[:], corr_t[:], ch9b, start=False, stop=True)
            biasvec = const.tile([IPC, 1], f32)
            nc.vector.tensor_scalar(biasvec[:], lini_ps[:], b1f, None, A_OP.add)

            # rbT = bf16(Wr @ e_allT + b0)
            rbT = big.tile([D, N], bf16)
            for hh in range(2):
                sl = slice(hh * H, (hh + 1) * H)
                rp_ps = ps.tile([D, H], f32, tag="ps")
                nc.tensor.matmul(rp_ps[:], wrT, eallT[:, sl], start=True, stop=True)
                nc.vector.tensor_scalar(rbT[:, sl], rp_ps[:], b0c, None, A_OP.add)

            # ---- main loop over interpolation levels ----------------
            # t1 = |B - q| for all q upfront on ACT so DVE never stalls on
            # the cross-engine dependency; G tiles split ACT/DVE; the
            # stop-q (15) G is produced early so the last accumulation has
            # no wait.
            z0 = psz.tile([D, H], f32, tag="z")
            z1 = psz.tile([D, H], f32, tag="z")

            t1s = []
            for q in range(Q):
                t1 = hold.tile([D, IPC], bf16, tag=f"t1_{q}")
                nc.scalar.activation(t1[:], B[:], AF.Abs, bias=nqc(q), scale=1.0)
                t1s.append(t1)

            # ---- c-branch: c_fs over all j --------------------------
            cl_ps = ps.tile([D, 1], f32, tag="ps")
            nc.tensor.matmul(cl_ps[:], wlT, cb, start=True, stop=True)
            cl = const.tile([D, 1], f32)
            nc.scalar.copy(cl[:], cl_ps[:])
            clb = const.tile([D, 1], bf16)
            nc.vector.tensor_copy(clb[:], cl_ps[:])
            clw_ps = ps.tile([1, 1], f32, tag="ps")
            nc.tensor.matmul(clw_ps[:], clb[:], w1b, start=True, stop=True)
            bc = const.tile([1, 1], f32)
            nc.vector.tensor_scalar(bc[:], clw_ps[:], 0.1, b1f, A_OP.mult, A_OP.add)

            cfs_row = const.tile([1, N], bf16)
            Ac = big.tile([D, N], bf16)
            nc.scalar.activation(Ac[:], rbT[:], AF.Relu, bias=cl[:], scale=1.0)
            for hh in range(2):
                sl = slice(hh * H, (hh + 1) * H)
                zc_ps = ps.tile([1, H], f32, tag="ps")
                nc.tensor.matmul(zc_ps[:], w9b, Ac[:, sl], start=True, stop=False)
                nc.tensor.matmul(zc_ps[:], w01b, rbT[:, sl], start=False, stop=True)
                nc.scalar.activation(
                    cfs_row[:, sl], zc_ps[:], AF.Sigmoid, bias=bc[:], scale=1.0
                )


            GACT = set(range(Q - 2 - N_GACT, Q - 2))  # ACT-built G levels
            gdve_order = [0, Q - 1] + [q for q in range(1, Q - 1) if q not in GACT]
            Gs = {}
            for q in GACT:
                G = hold.tile([D, N], bf16, tag=f"G_{q}")
                nc.scalar.activation(G[:], rbT[:], AF.Relu, bias=lqc(q), scale=1.0)
                Gs[q] = G

            def emit_sq(q):
                t2 = sw.tile([D, IPC], bf16, tag="t2")
                nc.vector.tensor_scalar(
                    t2[:], t1s[q][:], 1.0, 0.0, A_OP.subtract, A_OP.min
                )
                Sq = hold.tile([D, IPC], bf16, tag=f"Sq_{q}")
                nc.vector.tensor_scalar(Sq[:], t2[:], w9nc, None, A_OP.mult)
                return Sq

            def emit_gdve(q):
                G = hold.tile([D, N], bf16, tag=f"G_{q}")
                lq = LO + q * HSTEP
                nc.vector.tensor_scalar(
                    G[:], rbT[:], float(lq), 0.0, A_OP.add, A_OP.max
                )
                Gs[q] = G

            Sqs = {}
            gd = iter(gdve_order)
            for q in range(Q):
                Sqs[q] = emit_sq(q)
                nq = next(gd, None)
                if nq is not None:
                    emit_gdve(nq)

            for q in range(Q):
                st = q == 0
                sp = q == Q - 1
                nc.tensor.matmul(z0[:], Sqs[q][:], Gs[q][:, :H], start=st, stop=sp)
                nc.tensor.matmul(z1[:], Sqs[q][:], Gs[q][:, H:], start=st, stop=sp)
                if q == 0:
                    # fold 0.1*lin_j into every row (PSUM accumulation is
                    # order-independent)
                    nc.tensor.matmul(z0[:], w1rep, rbT[:, :H], start=False, stop=False)
                    nc.tensor.matmul(z1[:], w1rep, rbT[:, H:], start=False, stop=False)

            # crep broadcast AFTER the loop matmuls so the in-order PE
            # queue never stalls the z accumulation on the cfs sigmoids
            crep = []
            for hh in range(2):
                sl = slice(hh * H, (hh + 1) * H)
                cr = psc.tile([D, H], f32, tag="crep")
                nc.tensor.matmul(
                    cr[:], ones_row, cfs_row[0:1, sl], start=True, stop=True
                )
                crep.append(cr)

            # ---- epilogue (quarter-pipelined) ------------------------
            QH = H // 2
            rfs = big.tile([D, N], f32)
            prod = big.tile([D, N], f32)
            outc4 = const.tile([IPC, 4], f32)
            for qq in range(4):
                hh = qq // 2
                zb = (z0, z1)[hh]
                zsl = slice((qq % 2) * QH, (qq % 2) * QH + QH)
                sl = slice(qq * QH, (qq + 1) * QH)
                nc.scalar.activation(
                    rfs[:, sl], zb[:, zsl], AF.Sigmoid, bias=biasvec[:], scale=1.0
                )
                nc.vector.tensor_tensor(
                    prod[:, sl], rfs[:, sl], crep[hh][:, zsl], A_OP.mult
                )
                nc.vector.tensor_reduce(
                    outc4[:, qq : qq + 1],
                    prod[:, sl],
                    axis=mybir.AxisListType.X,
                    op=A_OP.max,
                )
            outc2 = const.tile([IPC, 2], f32)
            nc.vector.tensor_tensor(
                outc2[:], outc4[:, 0:2], outc4[:, 2:4], A_OP.max
            )
            outc = const.tile([IPC, 1], f32)
            nc.vector.tensor_tensor(
                outc[:], outc2[:, 0:1], outc2[:, 1:2], A_OP.max
            )
            # transpose [IPC, 1] -> [1, IPC] so the output DMA is one line
            orow_ps = ps.tile([1, IPC], f32, tag="ps")
            nc.tensor.matmul(orow_ps[:], outc[:], ident[:], start=True, stop=True)
            orow = const.tile([1, IPC], f32)
            nc.scalar.copy(orow[:], orow_ps[:])
            nc.sync.dma_start(d_out[:], orow[:])

    return nc


def _host_prep(anon_e_emb, e_table, c_emb, r_emb, W0, b0, W1, b1):
    f = np.float32
    bft = ml_dtypes.bfloat16
    anon_e_emb = np.asarray(anon_e_emb, f)
    e_table = np.asarray(e_table, f)
    c_emb = np.asarray(c_emb, f)
    r_emb = np.asarray(r_emb, f)
    W0 = np.asarray(W0, f)
    b0 = np.asarray(b0, f)
    W1 = np.asarray(W1, f)
    b1 = np.asarray(b1, f)

    Wl = W0[:, :D]
    e_all = np.concatenate([e_table, anon_e_emb], axis=0)  # [N, D]
    e_allT = np.ascontiguousarray(e_all.T)  # [D, N]

    bf_base = np.zeros((D, _BF_COLS), bft)
    bf_base[:, :N] = e_allT.astype(bft)
    bf_base[:, N : N + D] = Wl.T.astype(bft)
    bf_base[:, N + D : N + 2 * D] = W0[:, D:].T.astype(bft)
    bf_base[:, N + 2 * D : N + 3 * D] = np.tile(
        (0.1 * W1).astype(bft)[:, None], (1, D)
    )
    bf_base[:, _C0 + 0] = r_emb.astype(bft)
    bf_base[:, _C0 + 1] = b0.astype(bft)
    bf_base[:, _C0 + 2] = (-0.9 * W1).astype(bft)
    bf_base[:, _C0 + 3] = (0.1 * (W1 @ Wl)).astype(bft)
    bf_base[:, _C0 + 4] = (HSTEP * 0.9 * W1).astype(bft)
    bf_base[:, _C0 + 5] = c_emb.astype(bft)
    bf_base[:, _C0 + 6] = W1.astype(bft)
    bf_base[:, _C0 + 7] = (0.9 * W1).astype(bft)
    bf_base[:, _C0 + 8] = (0.1 * W1).astype(bft)
    for q in range(Q):
        bf_base[:, _CLQ + q] = np.float32(LO + q * HSTEP).astype(bft)
        bf_base[:, _CNQ + q] = np.float32(-q).astype(bft)

    rows = np.ones((1, D), bft)
    ident = np.eye(D, dtype=f)
    b1f = float(b1[0])

    in_maps = []
    for c in range(NCORES):
        bf_pack = bf_base.copy()
        bf_pack[:, _CMY:] = e_allT[:, c * IPC : (c + 1) * IPC].astype(bft)
        in_maps.append({"bf_pack": bf_pack, "rows": rows, "ident": ident})
    return in_maps, b1f


def _install_ntff_shim():
    """Provide antenv.axon_hooks (missing in this image) so that
    run_bass_kernel_spmd(trace=True) can collect NTFF profiles."""
    import sys
    import types

    if "antenv.axon_hooks" in sys.modules:
        return
    try:
        import antenv
        from trn_agent_boot.trn_boot import _ntff_profile_via_ctypes
    except ImportError:
        return
    mod = types.ModuleType("antenv.axon_hooks")
    state = {"hook": None}
    mod.set_axon_ntff_profile_hook = lambda h: state.__setitem__("hook", h)
    mod.get_axon_ntff_profile_hook = lambda: state["hook"]
    sys.modules["antenv.axon_hooks"] = mod
    antenv.axon_hooks = mod
    try:
        mod.set_axon_ntff_profile_hook(
            _ntff_profile_via_ctypes("/opt/axon/libaxon_pjrt.so")
        )
    except Exception:
        pass


def kernel_ex(inputs: dict, trace: bool = False):
    """Run on 8 NeuronCores; returns (out [N] float32, BassKernelResults)."""
    from concourse.bass_utils import run_bass_kernel_spmd

    if trace:
        _install_ntff_shim()

    in_maps, b1f = _host_prep(**inputs)
    key = (round(b1f, 10),)
    nc = _PROGRAM_CACHE.get(key)
    if nc is None:
        nc = _build_program(b1f)
        nc.finalize()
        _PROGRAM_CACHE[key] = nc

    res = run_bass_kernel_spmd(
        nc, in_maps, core_ids=list(range(NCORES)), trace=trace
    )
    out = np.concatenate(
        [
            np.asarray(res.results[c]["out"], np.float32).reshape(IPC)
            for c in range(NCORES)
        ]
    )
    return out, res


def kernel(**inputs) -> np.ndarray:
    out, _ = kernel_ex(inputs, trace=False)
    return out


# revision 37
# speedup vs baseline: 1.3718x; 1.0557x over previous
"""FALCON ObjectSomeValuesFrom forward kernel for Trainium2 (8 NeuronCores).

Math (reference):
    e_all = concat(e_table, anon_e_emb)            # [n, d], n=1024, d=128
    Wl, Wr = W0[:, :d], W0[:, d:]
    c_fs  = sigmoid(leaky(c@Wl.T + e_all@Wr.T + b0) @ W1 + b1)        # [n]
    left  = (e_all + r) @ Wl.T ; rp = e_all @ Wr.T + b0
    z_ij  = leaky(left_i + rp_j) @ W1                                  # [n, n]
    out_i = max_j sigmoid(z_ij + b1) * c_fs[j]

Algorithm (quantized interpolation): with leaky(x) = 0.1 x + 0.9 relu(x),
    z_ij = 0.1 (lin_i + lin_j) + sum_k w9_k relu(L_ik + rp_jk),  w9 = 0.9 W1.
Clip L to [-A, A] with A >= max|rp| and correct exactly:
    relu(L + rp) = relu(clip(L) + rp) + relu(L - A)    (the last term is
    j-independent -> folded into the per-i sigmoid bias).
Quantize clip(L) on a Q-level grid l_q, piecewise-LINEAR interp in L:
    relu(L + rp) ~= sum_q hat((L - lo)/h - q) * relu(l_q + rp)
so the [n, n, d] relu tensor is replaced by Q relu tables G_q = relu(rp + l_q)
([d, n] each, built by DVE/ACT) and interpolation weights
S_q[k, i] = w9_k hat(B_ik - q) ([d, n_i]; ACT Abs + 2 cheap DVE ops per q),
contracted on the PE: z_relu = sum_q S_q^T @ G_q (PSUM accumulation over q,
4-strip concurrent matmuls).  Elementwise work drops from n_i to Q tiles.

The output column [IPC, 1] is PE-transposed to a [1, IPC] row before the
store DMA (a partition-strided 4 B/line DMA costs ~8 us; one 512 B line is
cheap).

Sharding: i-rows split across 8 cores; e_table/weights/embeddings replicated;
final max over j is local per core.
"""

import numpy as np
import ml_dtypes

N = 1024
D = 128
NCORES = 8
IPC = N // NCORES  # i rows per core = 128
H = 512            # PSUM bank free size (fp32)

Q = 12             # interpolation levels
A_CLIP = 0.5       # L clip range; exact correction term added for L > A
LO = -A_CLIP
HSTEP = 2.0 * A_CLIP / (Q - 1)

N_GACT = 2         # G tiles built by ACT (rest on DVE)

_PROGRAM_CACHE: dict = {}

# bf16 pack layout (columns):
#   e_allT[1024] | wlT[128] | wrT[128] | w1rep01[128] |
#   rc | b0 | w9n | u01 | ch9 | c | w1 | w9 | w01 | lq[Q] | negq[Q] |
#   e_myT[128]
_C0 = N + 3 * D
_NSCAL = 9
_CLQ = _C0 + _NSCAL
_CNQ = _CLQ + Q
_CMY = _CNQ + Q
_BF_COLS = _CMY + D


def _build_program(b1f: float):
    import concourse.bacc as bacc
    import concourse.mybir as mybir
    import concourse.tile as tile

    f32 = mybir.dt.float32
    bf16 = mybir.dt.bfloat16
    A_OP = mybir.AluOpType
    AF = mybir.ActivationFunctionType

    nc = bacc.Bacc(None, target_bir_lowering=False, name="falcon_fwd")

    d_bf = nc.dram_tensor("bf_pack", [D, _BF_COLS], bf16, kind="ExternalInput")
    d_rows = nc.dram_tensor("rows", [1, D], bf16, kind="ExternalInput")
    d_ident = nc.dram_tensor("ident", [D, D], f32, kind="ExternalInput")
    d_out = nc.dram_tensor("out", [1, IPC], f32, kind="ExternalOutput")

    with tile.TileContext(nc) as tc:
        with (
            tc.tile_pool(name="const", bufs=1) as const,
            tc.tile_pool(name="big", bufs=1) as big,
            tc.tile_pool(name="sw", bufs=4) as sw,
            tc.tile_pool(name="hold", bufs=1) as hold,
            tc.tile_pool(name="ps", bufs=3, space="PSUM") as ps,
            tc.tile_pool(name="psz", bufs=2, space="PSUM") as psz,
            tc.tile_pool(name="psc", bufs=2, space="PSUM") as psc,
        ):
            # ---- input DMAs: weights/cols tail first, then e_allT -----
            bf = big.tile([D, _BF_COLS], bf16)
            rows_raw = const.tile([1, D], bf16)
            ident = const.tile([D, D], f32)
            # critical-path first: scalar cols + e_myT, then weights,
            # then the e_allT halves
            nc.sync.dma_start(bf[:, _C0:], d_bf[:, _C0:])
            nc.sync.dma_start(bf[:, N:_C0], d_bf[:, N:_C0])
            nc.sync.dma_start(bf[:, :H], d_bf[:, :H])
            nc.sync.dma_start(bf[:, H:N], d_bf[:, H:N])
            nc.sync.dma_start(rows_raw[:], d_rows[:])
            nc.sync.dma_start(ident[:], d_ident[:])

            # dummy sigmoid first: forces the one ACT table set that
            # contains {sigmoid, abs, relu, copy} to load exactly once
            dum = const.tile([1, 1], f32)
            nc.vector.memset(dum[:], 0.0)
            nc.scalar.activation(dum[:], dum[:], AF.Sigmoid)

            # funnel the scalar-pointer columns through one DVE copy
            # (bf16 in the DMA pack for wide lines; f32 on-chip for ptr ops)
            colsB = const.tile([D, _NSCAL + 2 * Q], f32)
            nc.vector.tensor_copy(colsB[:], bf[:, _C0:_CMY])
            rowsS = const.tile([1, D], bf16)
            nc.vector.tensor_copy(rowsS[:], rows_raw[:])

            rc = colsB[:, 0:1]
            b0c = colsB[:, 1:2]
            w9nc = colsB[:, 2:3]
            # matmul rhs columns must be bf16 -> slice the DMA'd pack
            u01b = bf[:, _C0 + 3 : _C0 + 4]
            ch9b = bf[:, _C0 + 4 : _C0 + 5]
            cb = bf[:, _C0 + 5 : _C0 + 6]
            w1b = bf[:, _C0 + 6 : _C0 + 7]
            w9b = bf[:, _C0 + 7 : _C0 + 8]
            w01b = bf[:, _C0 + 8 : _C0 + 9]
            lqc = lambda q: colsB[:, _NSCAL + q : _NSCAL + q + 1]
            nqc = lambda q: colsB[:, _NSCAL + Q + q : _NSCAL + Q + q + 1]
            eallT = bf[:, :N]
            wlT = bf[:, N : N + D]
            wrT = bf[:, N + D : N + 2 * D]
            w1rep = bf[:, N + 2 * D : N + 3 * D]
            emyT = bf[:, _CMY:]
            ones_row = rowsS[:, :]

            # ---- prologue -------------------------------------------
            # er_myT = e_myT + r (bf16), left via PE, B = clip((L-lo)/h)
            er_myT = const.tile([D, IPC], bf16)
            nc.vector.tensor_scalar(er_myT[:], emyT, rc, None, A_OP.add)
            left_ps = ps.tile([D, IPC], f32, tag="ps")
            nc.tensor.matmul(left_ps[:], wlT, er_myT[:], start=True, stop=True)
            B1 = const.tile([D, IPC], f32)
            nc.vector.tensor_scalar(
                B1[:], left_ps[:], 1.0 / HSTEP, -LO / HSTEP, A_OP.mult, A_OP.add
            )
            B = const.tile([D, IPC], f32)
            nc.vector.tensor_scalar(B[:], B1[:], 0.0, Q - 1.0, A_OP.max, A_OP.min)
            corr_t = const.tile([D, IPC], bf16)
            nc.vector.tensor_scalar(
                corr_t[:], B1[:], Q - 1.0, 0.0, A_OP.subtract, A_OP.max
            )
            # bias accumulation: 0.1*lin_i + corr_i  (then + b1)
            lini_ps = ps.tile([IPC, 1], f32, tag="ps")
            nc.tensor.matmul(lini_ps[:], er_myT[:], u01b, start=True, stop=False)
            nc.tensor.matmul(lini_ps[:], corr_t[:], ch9b, start=False, stop=True)
            biasvec = const.tile([IPC, 1], f32)
            nc.vector.tensor_scalar(biasvec[:], lini_ps[:], b1f, None, A_OP.add)

            # rbT = bf16(Wr @ e_allT + b0)
            rbT = big.tile([D, N], bf16)
            for hh in range(2):
                sl = slice(hh * H, (hh + 1) * H)
                rp_ps = ps.tile([D, H], f32, tag="ps")
                nc.tensor.matmul(rp_ps[:], wrT, eallT[:, sl], start=True, stop=True)
                nc.vector.tensor_scalar(rbT[:, sl], rp_ps[:], b0c, None, A_OP.add)

            # ---- main loop over interpolation levels ----------------
            # t1 = |B - q| for all q upfront on ACT so DVE never stalls on
            # the cross-engine dependency; G tiles split ACT/DVE; the
            # stop-q (15) G is produced early so the last accumulation has
            # no wait.
            z0 = psz.tile([D, H], f32, tag="z")
            z1 = psz.tile([D, H], f32, tag="z")

            t1s = []
            for q in range(Q):
                t1 = hold.tile([D, IPC], bf16, tag=f"t1_{q}")
                nc.scalar.activation(t1[:], B[:], AF.Abs, bias=nqc(q), scale=1.0)
                t1s.append(t1)

            # ---- c-branch: c_fs over all j --------------------------
            cl_ps = ps.tile([D, 1], f32, tag="ps")
            nc.tensor.matmul(cl_ps[:], wlT, cb, start=True, stop=True)
            cl = const.tile([D, 1], f32)
            nc.scalar.copy(cl[:], cl_ps[:])
            clb = const.tile([D, 1], bf16)
            nc.vector.tensor_copy(clb[:], cl_ps[:])
            clw_ps = ps.tile([1, 1], f32, tag="ps")
            nc.tensor.matmul(clw_ps[:], clb[:], w1b, start=True, stop=True)
            bc = const.tile([1, 1], f32)
            nc.vector.tensor_scalar(bc[:], clw_ps[:], 0.1, b1f, A_OP.mult, A_OP.add)

            cfs_row = const.tile([1, N], bf16)
            Ac = big.tile([D, N], bf16)
            nc.scalar.activation(Ac[:], rbT[:], AF.Relu, bias=cl[:], scale=1.0)
            for hh in range(2):
                sl = slice(hh * H, (hh + 1) * H)
                zc_ps = ps.tile([1, H], f32, tag="ps")
                nc.tensor.matmul(zc_ps[:], w9b, Ac[:, sl], start=True, stop=False)
                nc.tensor.matmul(zc_ps[:], w01b, rbT[:, sl], start=False, stop=True)
                nc.scalar.activation(
                    cfs_row[:, sl], zc_ps[:], AF.Sigmoid, bias=bc[:], scale=1.0
                )


            GACT = set(range(Q - 2 - N_GACT, Q - 2))  # ACT-built G levels
            gdve_order = [0, Q - 1] + [q for q in range(1, Q - 1) if q not in GACT]
            Gs = {}
            for q in GACT:
                G = hold.tile([D, N], bf16, tag=f"G_{q}")
                nc.scalar.activation(G[:], rbT[:], AF.Relu, bias=lqc(q), scale=1.0)
                Gs[q] = G

            def emit_sq(q):
                t2 = sw.tile([D, IPC], bf16, tag="t2")
                nc.vector.tensor_scalar(
                    t2[:], t1s[q][:], 1.0, 0.0, A_OP.subtract, A_OP.min
                )
                Sq = hold.tile([D, IPC], bf16, tag=f"Sq_{q}")
                nc.vector.tensor_scalar(Sq[:], t2[:], w9nc, None, A_OP.mult)
                return Sq

            def emit_gdve(q):
                G = hold.tile([D, N], bf16, tag=f"G_{q}")
                lq = LO + q * HSTEP
                nc.vector.tensor_scalar(
                    G[:], rbT[:], float(lq), 0.0, A_OP.add, A_OP.max
                )
                Gs[q] = G

            Sqs = {}
            gd = iter(gdve_order)
            for q in range(Q):
                Sqs[q] = emit_sq(q)
                nq = next(gd, None)
                if nq is not None:
                    emit_gdve(nq)

            for q in range(Q):
                st = q == 0
                sp = q == Q - 1
                nc.tensor.matmul(z0[:], Sqs[q][:], Gs[q][:, :H], start=st, stop=sp)
                nc.tensor.matmul(z1[:], Sqs[q][:], Gs[q][:, H:], start=st, stop=sp)
                if q == 0:
                    # fold 0.1*lin_j into every row (PSUM accumulation is
                    # order-independent)
                    nc.tensor.matmul(z0[:], w1rep, rbT[:, :H], start=False, stop=False)
                    nc.tensor.matmul(z1[:], w1rep, rbT[:, H:], start=False, stop=False)

            # crep broadcast AFTER the loop matmuls so the in-order PE
            # queue never stalls the z accumulation on the cfs sigmoids
            crep = []
            for hh in range(2):
                sl = slice(hh * H, (hh + 1) * H)
                cr = psc.tile([D, H], f32, tag="crep")
                nc.tensor.matmul(
                    cr[:], ones_row, cfs_row[0:1, sl], start=True, stop=True
                )
                crep.append(cr)

            # ---- epilogue (quarter-pipelined) ------------------------
            QH = H // 2
            rfs = big.tile([D, N], f32)
            prod = big.tile([D, N], f32)
            outc4 = const.tile([IPC, 4], f32)
            for qq in range(4):
                hh = qq // 2
                zb = (z0, z1)[hh]
                zsl = slice((qq % 2) * QH, (qq % 2) * QH + QH)
                sl = slice(qq * QH, (qq + 1) * QH)
                nc.scalar.activation(
                    rfs[:, sl], zb[:, zsl], AF.Sigmoid, bias=biasvec[:], scale=1.0
                )
                nc.vector.tensor_tensor(
                    prod[:, sl], rfs[:, sl], crep[hh][:, zsl], A_OP.mult
                )
                nc.vector.tensor_reduce(
                    outc4[:, qq : qq + 1],
                    prod[:, sl],
                    axis=mybir.AxisListType.X,
                    op=A_OP.max,
                )
            outc = const.tile([IPC, 1], f32)
            nc.vector.tensor_reduce(
                outc[:], outc4[:], axis=mybir.AxisListType.X, op=A_OP.max
            )
            # transpose [IPC, 1] -> [1, IPC] so the output DMA is one line
            orow_ps = ps.tile([1, IPC], f32, tag="ps")
            nc.tensor.matmul(orow_ps[:], outc[:], ident[:], start=True, stop=True)
            orow = const.tile([1, IPC], f32)
            nc.scalar.copy(orow[:], orow_ps[:])
            nc.sync.dma_start(d_out[:], orow[:])

    return nc


def _host_prep(anon_e_emb, e_table, c_emb, r_emb, W0, b0, W1, b1):
    f = np.float32
    bft = ml_dtypes.bfloat16
    anon_e_emb = np.asarray(anon_e_emb, f)
    e_table = np.asarray(e_table, f)
    c_emb = np.asarray(c_emb, f)
    r_emb = np.asarray(r_emb, f)
    W0 = np.asarray(W0, f)
    b0 = np.asarray(b0, f)
    W1 = np.asarray(W1, f)
    b1 = np.asarray(b1, f)

    Wl = W0[:, :D]
    e_all = np.concatenate([e_table, anon_e_emb], axis=0)  # [N, D]
    e_allT = np.ascontiguousarray(e_all.T)  # [D, N]

    bf_base = np.zeros((D, _BF_COLS), bft)
    bf_base[:, :N] = e_allT.astype(bft)
    bf_base[:, N : N + D] = Wl.T.astype(bft)
    bf_base[:, N + D : N + 2 * D] = W0[:, D:].T.astype(bft)
    bf_base[:, N + 2 * D : N + 3 * D] = np.tile(
        (0.1 * W1).astype(bft)[:, None], (1, D)
    )
    bf_base[:, _C0 + 0] = r_emb.astype(bft)
    bf_base[:, _C0 + 1] = b0.astype(bft)
    bf_base[:, _C0 + 2] = (-0.9 * W1).astype(bft)
    bf_base[:, _C0 + 3] = (0.1 * (W1 @ Wl)).astype(bft)
    bf_base[:, _C0 + 4] = (HSTEP * 0.9 * W1).astype(bft)
    bf_base[:, _C0 + 5] = c_emb.astype(bft)
    bf_base[:, _C0 + 6] = W1.astype(bft)
    bf_base[:, _C0 + 7] = (0.9 * W1).astype(bft)
    bf_base[:, _C0 + 8] = (0.1 * W1).astype(bft)
    for q in range(Q):
        bf_base[:, _CLQ + q] = np.float32(LO + q * HSTEP).astype(bft)
        bf_base[:, _CNQ + q] = np.float32(-q).astype(bft)

    rows = np.ones((1, D), bft)
    ident = np.eye(D, dtype=f)
    b1f = float(b1[0])

    in_maps = []
    for c in range(NCORES):
        bf_pack = bf_base.copy()
        bf_pack[:, _CMY:] = e_allT[:, c * IPC : (c + 1) * IPC].astype(bft)
        in_maps.append({"bf_pack": bf_pack, "rows": rows, "ident": ident})
    return in_maps, b1f


def _install_ntff_shim():
    """Provide antenv.axon_hooks (missing in this image) so that
    run_bass_kernel_spmd(trace=True) can collect NTFF profiles."""
    import sys
    import types

    if "antenv.axon_hooks" in sys.modules:
        return
    try:
        import antenv
        from trn_agent_boot.trn_boot import _ntff_profile_via_ctypes
    except ImportError:
        return
    mod = types.ModuleType("antenv.axon_hooks")
    state = {"hook": None}
    mod.set_axon_ntff_profile_hook = lambda h: state.__setitem__("hook", h)
    mod.get_axon_ntff_profile_hook = lambda: state["hook"]
    sys.modules["antenv.axon_hooks"] = mod
    antenv.axon_hooks = mod
    try:
        mod.set_axon_ntff_profile_hook(
            _ntff_profile_via_ctypes("/opt/axon/libaxon_pjrt.so")
        )
    except Exception:
        pass


def kernel_ex(inputs: dict, trace: bool = False):
    """Run on 8 NeuronCores; returns (out [N] float32, BassKernelResults)."""
    from concourse.bass_utils import run_bass_kernel_spmd

    if trace:
        _install_ntff_shim()

    in_maps, b1f = _host_prep(**inputs)
    key = (round(b1f, 10),)
    nc = _PROGRAM_CACHE.get(key)
    if nc is None:
        nc = _build_program(b1f)
        nc.finalize()
        _PROGRAM_CACHE[key] = nc

    res = run_bass_kernel_spmd(
        nc, in_maps, core_ids=list(range(NCORES)), trace=trace
    )
    out = np.concatenate(
        [
            np.asarray(res.results[c]["out"], np.float32).reshape(IPC)
            for c in range(NCORES)
        ]
    )
    return out, res


def kernel(**inputs) -> np.ndarray:
    out, _ = kernel_ex(inputs, trace=False)
    return out
